# revision 1
# baseline (speedup 1.0000x reference)
"""Trainium2 Bass kernel for nn_DSGSF (batched bidirectional multi-scale LSTM).

Strategy: pure data parallel over 8 cores. Everything on-device runs in a
transposed layout: feature/gate rows on SBUF partitions, samples on the free
dim. Host pre-packs all weights:
  - per-(dir,level,gate) block-diagonal lhsT matrices so one matmul computes a
    gate type for all steps of a level (K=128, M=64);
  - hh lhsT matrices with jnp.repeat folded in;
  - conv1d band matrices (with the 1/4 exp prescale folded in);
  - block-ones matrices that compute softmax denominators broadcast back onto
    the exp rows in a single matmul.
Sigmoid/tanh run on ACT (one table set), exp runs as a custom fused DVE op
(cubic poly + two squarings), reciprocal via RECIPROCAL_APPROX_FAST.
"""

import numpy as np
import ml_dtypes

B, D, H = 131072, 128, 64
NCORES = 8
S = B // NCORES          # samples per core
NT = 512                 # samples per tile (one PSUM bank)

GATES = ("i", "f", "g", "o")

# ---------------------------------------------------------------------------
# exp4 polynomial: exp(y) = p(y/4)^4 with p(u) = 1 + c1 u + c2 u^2 + c3 u^3
# fitted for relative error of p(u)^4 vs e^(4u) on |u| <= EXP_FIT_RANGE.
EXP_FIT_RANGE = 0.85


def _fit_exp_poly():
    u = np.linspace(-EXP_FIT_RANGE, EXP_FIT_RANGE, 4001)
    # weighted least squares: minimize sum ((p(u) - e^u)/e^u)^2
    A = np.stack([u, u * u, u**3], axis=1) / np.exp(u)[:, None]
    b = (np.exp(u) - 1.0) / np.exp(u)
    c, *_ = np.linalg.lstsq(A, b, rcond=None)
    return c  # c1, c2, c3


EXP_C1, EXP_C2, EXP_C3 = (float(v) for v in _fit_exp_poly())


def _fit_tanh5():
    # odd degree-5 fit t(u) = u*(a + b u^2 + c u^4) ~ tanh(u) on [-2.05, 2.05]
    u = np.linspace(-2.05, 2.05, 4001)
    A = np.stack([u, u**3, u**5], axis=1)
    c, *_ = np.linalg.lstsq(A, np.tanh(u), rcond=None)
    return c


TANH_A, TANH_B, TANH_C = (float(v) for v in _fit_tanh5())


def tanh5_np(u):
    return u * (TANH_A + TANH_B * u * u + TANH_C * u**4)


def exp4_np(y):
    u = y.astype(np.float64)  # y already prescaled by 1/4 in conv matrices
    p = 1.0 + u * (EXP_C1 + u * (EXP_C2 + EXP_C3 * u))
    return (p * p) ** 2


def recip_np(x):
    # mirror of RECIPROCAL_APPROX_FAST (fp32 bit-flip seed + 2 NR passes)
    x = x.astype(np.float32)
    nx = (~x.view(np.int32)).view(np.float32)
    y0 = nx * np.float32(-0.23549792)
    y1 = y0 * (np.float32(2.0017324) - x * y0)
    return y1 * (np.float32(2.0) - x * y1)


def _sigmoid(x):
    return 1.0 / (1.0 + np.exp(-x))


# ---------------------------------------------------------------------------
# Host-side weight packing.
#
# Level parameters: level l in 1..4 has S_l steps, dh = 64/S_l, di = 128/S_l.
LEVEL_STEPS = {1: 1, 2: 2, 3: 4, 4: 8}


def _ih_lhsT(w_ih, level, gate):
    """Block-diagonal lhsT [128, 64] computing `gate` preacts for all steps."""
    Sl = LEVEL_STEPS[level]
    dh, di = 64 // Sl, 128 // Sl
    gi = GATES.index(gate)
    wg = w_ih[gi * dh:(gi + 1) * dh, :]          # [dh, di]
    out = np.zeros((128, 64), np.float32)
    for s in range(Sl):
        out[s * di:(s + 1) * di, s * dh:(s + 1) * dh] = wg.T
    return out


def _hh_lhsT(w_hh, level, gate, direction):
    """lhsT [K, 64] mapping the hb rows onto gate preacts for all steps.

    fwd level l uses hb from level l-1 with jnp.repeat semantics:
      l=2: hb has 32 rows (one block), both steps use it.
      l=3: hb2 has 2 blocks of 16; step s uses block s//2.
      l=4: hb3 has 4 blocks of 8; step s uses block s//2.
    bwd level l uses hb from level l+1 with 1:1 step mapping (K = 64),
    except bwd l=1 (1 step, hb has 64 rows).
    """
    Sl = LEVEL_STEPS[level]
    dh = 64 // Sl
    dprev = w_hh.shape[1]                         # prev-level dh
    gi = GATES.index(gate)
    wg = w_hh[gi * dh:(gi + 1) * dh, :]           # [dh, dprev]
    if direction == "f":
        nblk = {2: 1, 3: 2, 4: 4}[level]
        K = nblk * dprev
        out = np.zeros((K, 64), np.float32)
        for s in range(Sl):
            blk = s * nblk // Sl                  # s//2 for l=3,4; 0 for l=2
            out[blk * dprev:(blk + 1) * dprev, s * dh:(s + 1) * dh] = wg.T
    else:
        K = Sl * dprev
        out = np.zeros((K, 64), np.float32)
        for s in range(Sl):
            out[s * dprev:(s + 1) * dprev, s * dh:(s + 1) * dh] = wg.T
    return out


def _conv_lhsT(w3, win, nblk, stride, scale):
    """Band matrix [nblk*win, nblk*wout] for blockwise conv1d(k=3,pad=1)."""
    w = np.asarray(w3, np.float64).reshape(3) * scale
    wout = win // stride
    blk = np.zeros((win, wout), np.float64)
    for j in range(wout):
        for t in range(3):
            k = stride * j - 1 + t
            if 0 <= k < win:
                blk[k, j] += w[t]
    out = np.zeros((nblk * win, nblk * wout), np.float32)
    for q in range(nblk):
        out[q * win:(q + 1) * win, q * wout:(q + 1) * wout] = blk
    return out


def _ones_block(width):
    out = np.zeros((128, 128), np.float32)
    for q in range(128 // width):
        out[q * width:(q + 1) * width, q * width:(q + 1) * width] = 1.0
    return out


def _bias_pair(b, level, gate):
    """[128] bias rows for a pair-packed gate psum tile (same 64 twice)."""
    Sl = LEVEL_STEPS[level]
    dh = 64 // Sl
    gi = GATES.index(gate)
    bg = np.asarray(b, np.float32)[gi * dh:(gi + 1) * dh]
    one = np.tile(bg, Sl)                          # [64], step-major
    return np.concatenate([one, one])


class Pack:
    """All host-packed constant matrices + column offset maps."""

    def __init__(self, inp):
        g = lambda n: np.asarray(inp[n], np.float32)

        # ---- ih lhsT blocks [128, 64] each
        self.wih = {}
        order = []
        for d in ("f", "b"):
            for lvl in (1, 2, 3, 4):
                w = g(f"{d}w_ih{lvl}")
                first = (d == "f" and lvl == 1) or (d == "b" and lvl == 4)
                last = d == "b" and lvl == 1
                gates = ("i", "g", "o") if first else (("i", "f", "g") if last else GATES)
                for gt in gates:
                    self.wih[f"{d}{lvl}_{gt}"] = _ih_lhsT(w, lvl, gt)
                    order.append(f"{d}{lvl}_{gt}")
        self.wih_order = order
        self.wih_mat = np.concatenate([self.wih[k] for k in order], axis=1)

        # ---- hh lhsT blocks [K, 64].  Stored in a [128, ...] matrix with the
        # block replicated at partition offset 64 (matmul requires lhsT and
        # rhs to share base_partition; the B-tile rhs lives at partitions 64+).
        self.whh = {}
        horder = []
        for lvl in (2, 3, 4):
            w = g(f"fw_hh{lvl}")
            for gt in GATES:
                self.whh[f"f{lvl}_{gt}"] = _hh_lhsT(w, lvl, gt, "f")
                horder.append(f"f{lvl}_{gt}")
        for lvl in (3, 2, 1):
            w = g(f"bw_hh{lvl}")
            gates = ("i", "f", "g") if lvl == 1 else GATES
            for gt in gates:
                self.whh[f"b{lvl}_{gt}"] = _hh_lhsT(w, lvl, gt, "b")
                horder.append(f"b{lvl}_{gt}")
        self.whh_order = horder
        self.whh_K = {k: self.whh[k].shape[0] for k in horder}
        mats = []
        for k in horder:
            m = np.zeros((128, 64), np.float32)
            m[: self.whh_K[k], :] = self.whh[k]
            m[64:64 + self.whh_K[k], :] = self.whh[k]
            mats.append(m)
        self.whh_mat = np.concatenate(mats, axis=1)

        # ---- conv band matrices (with exp 1/4 prescale folded in)
        sc = 0.25
        self.wconv = {
            "f12h": _conv_lhsT(g("ft12h"), 64, 1, 2, sc),
            "f12c": _conv_lhsT(g("ft12c"), 64, 1, 2, sc),
            "f23h": _conv_lhsT(g("ft23h"), 32, 2, 2, sc),
            "f23c": _conv_lhsT(g("ft23c"), 32, 2, 2, sc),
            "f34h": _conv_lhsT(g("ft34h"), 16, 4, 2, sc),
            # NB: reference reuses ft34h for the c path (original model bug)
            "f34c": _conv_lhsT(g("ft34h"), 16, 4, 2, sc),
            "b43h": _conv_lhsT(g("bt43h"), 16, 4, 1, sc),
            "b43c": _conv_lhsT(g("bt43c"), 16, 4, 1, sc),
            "b32h": _conv_lhsT(g("bt32h"), 32, 2, 1, sc),
            "b32c": _conv_lhsT(g("bt32c"), 32, 2, 1, sc),
            "b21h": _conv_lhsT(g("bt21h"), 64, 1, 1, sc),
            "b21c": _conv_lhsT(g("bt21c"), 64, 1, 1, sc),
        }
        self.conv_order = list(self.wconv.keys())
        # replicate at partition offset 64 for B-tile matmuls (see whh note)
        cmats = []
        for k in self.conv_order:
            c = self.wconv[k]
            m = np.zeros((128, c.shape[1]), np.float32)
            m[:64] = c
            m[64:] = c
            cmats.append(m)
        self.wconv_mat = np.concatenate(cmats, axis=1)

        # ---- ones block matrices
        self.wones = {w: _ones_block(w) for w in (8, 16, 32, 64)}
        self.ones_order = [8, 16, 32, 64]
        self.wones_mat = np.concatenate(
            [self.wones[w] for w in self.ones_order], axis=1)

        # ---- biases (pair-packed [128] cols)
        self.bias = {}
        border = []
        for d in ("f", "b"):
            for lvl in (1, 2, 3, 4):
                b = g(f"{d}b{lvl}")
                first = (d == "f" and lvl == 1) or (d == "b" and lvl == 4)
                last = d == "b" and lvl == 1
                gates = ("i", "g", "o") if first else (("i", "f", "g") if last else GATES)
                for gt in gates:
                    self.bias[f"{d}{lvl}_{gt}"] = _bias_pair(b, lvl, gt)
                    border.append(f"{d}{lvl}_{gt}")
        self.bias_order = border
        self.bias_mat = np.stack([self.bias[k] for k in border], axis=1)

        # offset maps (columns)
        self.wih_off = {k: 64 * i for i, k in enumerate(order)}
        self.whh_off = {k: 64 * i for i, k in enumerate(horder)}
        off = {}
        c = 0
        for k in self.conv_order:
            off[k] = c
            c += self.wconv[k].shape[1]
        self.conv_off = off
        self.ones_off = {w: 128 * i for i, w in enumerate(self.ones_order)}
        self.bias_off = {k: i for i, k in enumerate(border)}


# ---------------------------------------------------------------------------
# Numpy mirror of the device program (single "tile" covering all samples,
# PO=0 everywhere). Used to validate the packing before touching hardware.

def mirror_forward(pack: Pack, specT, exact=False):
    """specT: [128, n] float32. Returns outT [128, n]."""
    X = specT.astype(np.float32)
    myexp = (lambda y: np.exp(4.0 * y)) if exact else exp4_np
    myrecip = (lambda x: 1.0 / x) if exact else recip_np

    def mm(lhsT, rhs):
        return lhsT.T.astype(np.float32) @ rhs.astype(np.float32)

    def gates_ih(d, lvl, rhs):
        return {gt: mm(pack.wih[f"{d}{lvl}_{gt}"], rhs)
                for gt in GATES
                if f"{d}{lvl}_{gt}" in pack.wih}

    def add_hh(G, d, lvl, hb):
        for gt in list(G):
            G[gt] = G[gt] + mm(pack.whh[f"{d}{lvl}_{gt}"], hb)

    def add_bias(G, d, lvl):
        for gt in list(G):
            G[gt] = G[gt] + pack.bias[f"{d}{lvl}_{gt}"][:64, None]

    def trans(name_h, name_c, h, c, widths):
        eh = myexp(mm(pack.wconv[name_h], h))
        ec = myexp(mm(pack.wconv[name_c], c))
        e = np.concatenate([eh, ec], axis=0)        # [2*wout_total, n]
        ones = pack.wones[widths][: e.shape[0], : e.shape[0]]
        d_bc = mm(ones, e)
        inv = myrecip(d_bc.astype(np.float32))
        nb = e * inv
        half = eh.shape[0]
        return nb[:half], nb[half:]

    # ---------------- forward chain
    G = gates_ih("f", 1, X); add_bias(G, "f", 1)
    sI, sO, tG = _sigmoid(G["i"]), _sigmoid(G["o"]), np.tanh(G["g"])
    c1 = sI * tG
    h1 = sO * np.tanh(c1)
    hb1, cb1 = trans("f12h", "f12c", h1, c1, 32)

    G = gates_ih("f", 2, X); add_hh(G, "f", 2, hb1); add_bias(G, "f", 2)
    cb1d = np.concatenate([cb1, cb1], axis=0)       # 2 steps of 32
    t1 = _sigmoid(G["i"]) * np.tanh(G["g"])
    c2 = _sigmoid(G["f"]) * cb1d + t1
    h2 = _sigmoid(G["o"]) * np.tanh(c2)
    hb2, cb2 = trans("f23h", "f23c", h2, c2, 16)

    G = gates_ih("f", 3, X); add_hh(G, "f", 3, hb2); add_bias(G, "f", 3)
    # cb2 has 2 blocks of 16; step s of level 3 uses block s//2
    cb2d = np.concatenate([cb2[0:16], cb2[0:16], cb2[16:32], cb2[16:32]], axis=0)
    t1 = _sigmoid(G["i"]) * np.tanh(G["g"])
    c3 = _sigmoid(G["f"]) * cb2d + t1
    h3 = _sigmoid(G["o"]) * np.tanh(c3)
    hb3, cb3 = trans("f34h", "f34c", h3, c3, 8)

    G = gates_ih("f", 4, X); add_hh(G, "f", 4, hb3); add_bias(G, "f", 4)
    cb3d = np.concatenate(
        [cb3[8 * (s // 2):8 * (s // 2) + 8] for s in range(8)], axis=0)
    t1 = _sigmoid(G["i"]) * np.tanh(G["g"])
    c4 = _sigmoid(G["f"]) * cb3d + t1
    h4 = _sigmoid(G["o"]) * np.tanh(c4)             # -> out rows 0:64

    # ---------------- backward chain
    G = gates_ih("b", 4, X); add_bias(G, "b", 4)
    c4b = _sigmoid(G["i"]) * np.tanh(G["g"])
    h4b = _sigmoid(G["o"]) * np.tanh(c4b)
    hb4, cb4 = trans("b43h", "b43c", h4b, c4b, 16)

    G = gates_ih("b", 3, X); add_hh(G, "b", 3, hb4); add_bias(G, "b", 3)
    t1 = _sigmoid(G["i"]) * np.tanh(G["g"])
    c3b = _sigmoid(G["f"]) * cb4 + t1
    h3b = _sigmoid(G["o"]) * np.tanh(c3b)
    hb3b, cb3b = trans("b32h", "b32c", h3b, c3b, 32)

    G = gates_ih("b", 2, X); add_hh(G, "b", 2, hb3b); add_bias(G, "b", 2)
    t1 = _sigmoid(G["i"]) * np.tanh(G["g"])
    c2b = _sigmoid(G["f"]) * cb3b + t1
    h2b = _sigmoid(G["o"]) * np.tanh(c2b)
    hb2b, cb2b = trans("b21h", "b21c", h2b, c2b, 64)

    G = gates_ih("b", 1, X); add_hh(G, "b", 1, hb2b); add_bias(G, "b", 1)
    c1b = _sigmoid(G["f"]) * cb2b + _sigmoid(G["i"]) * np.tanh(G["g"])

    return np.concatenate([h4, c1b], axis=0)


# ---------------------------------------------------------------------------
# Device kernel (Bass / Tile).

_BUILD_CACHE = {}

# pool-size knobs (overridable before _build for tuning)
POOLCFG = {"pgf": 3, "pgb": 3, "pc": 1, "pd": 1, "spool": 6, "xpool": 16,
           "norm_eng": "vector", "group": 8}


def _register_exp4():
    """Custom fused DVE op: out = (1 + u(c1 + u(c2 + c3 u)))^4, u = in0."""
    import re
    import concourse.dve_ops as dve_ops
    from concourse.dve_ops import DveOp
    from concourse.dve_spec import Spec, Src0, C0, C1, C2, One

    for op in dve_ops.OPS:
        if op.name == "EXP4_ANT":
            return op
    u = Src0
    inner = C1 + C2 * u
    inner2 = C0 + u * inner
    p = One + u * inner2
    sq = p * p
    spec = Spec(
        body=sq * sq,
        reference=lambda in0, in1, s0, s1, imm2:
            (1.0 + in0 * (s0 + in0 * (s1 + imm2 * in0))) ** 4,
    )
    op = DveOp("EXP4_ANT", spec, subdim=False, uops_sha={})
    _register_op(op)
    return op


def _register_op(op):
    import re
    import concourse.dve_ops as dve_ops

    dve_ops.OPS.append(op)
    dve_ops._SUB_OPCODE_FOR_NAME[op.name] = (
        dve_ops._CUSTOM_DVE_ROW_BASE + len(dve_ops.OPS) - 1)
    dve_ops.CUSTOM_DVE_SPECS[op.name] = op.spec
    for ver in ("v3",):
        try:
            op.compile(ver)
        except ValueError as e:
            m = re.search(rf"\({ver}: ([0-9a-f]+)", str(e))
            if not m:
                raise
            op.uops_sha[ver] = m.group(1)
            op.compile(ver)


def _register_recip_mul():
    """out = in1 * recip1(in0): bit-flip seed + one NR pass (~0.4% rel)."""
    import concourse.dve_ops as dve_ops
    from concourse.dve_ops import DveOp
    from concourse.dve_spec import Spec, Src0, Src1, C0, C1, Bin, AluOp

    for op in dve_ops.OPS:
        if op.name == "RECIP1_MUL_ANT":
            return op

    def _ref(in0, in1, s0, s1, imm2):
        x = np.ascontiguousarray(in0, dtype=np.float32)
        nx = (~x.view(np.int32)).view(np.float32)
        y0 = nx * np.float32(s0)
        y1 = y0 * (np.float32(s1) - x * y0)
        return y1 * in1

    nx = Bin(AluOp.BITWISE_NOT, Src0, Src0)
    y0 = nx * C0
    y1 = y0 * (C1 - Src0 * y0)
    spec = Spec(body=y1 * Src1, reference=_ref)
    op = DveOp("RECIP1_MUL_ANT", spec, subdim=False, uops_sha={})
    _register_op(op)
    return op


def _register_tanh_mul():
    """out = in1 * tanh5(in0): fused h = sigma(o) * tanh(c)."""
    import concourse.dve_ops as dve_ops
    from concourse.dve_ops import DveOp
    from concourse.dve_spec import Spec, Src0, Src1, C0, C1, C2

    for op in dve_ops.OPS:
        if op.name == "TANH_MUL_ANT":
            return op
    u = Src0
    x2 = u * u
    x4 = x2 * x2
    t = u * (C0 + C1 * x2 + C2 * x4)
    spec = Spec(
        body=t * Src1,
        reference=lambda in0, in1, s0, s1, imm2:
            in0 * (s0 + s1 * in0 * in0 + imm2 * in0**4) * in1,
    )
    op = DveOp("TANH_MUL_ANT", spec, subdim=False, uops_sha={})
    _register_op(op)
    return op


def _build(n_samples):
    """Build + compile the Bacc program for one core processing n_samples."""
    key = (n_samples, tuple(sorted(POOLCFG.items())))
    if key in _BUILD_CACHE:
        return _BUILD_CACHE[key]

    import concourse.bass as bass
    import concourse.mybir as mybir
    from concourse import bacc
    from concourse.tile import TileContext
    from concourse.dve_ops import RECIPROCAL_APPROX_FAST, RECIP_APPROX_FAST_CONSTS

    EXP4 = _register_exp4()
    TMUL = _register_tanh_mul()
    RMUL = _register_recip_mul()
    RC = RECIP_APPROX_FAST_CONSTS

    bf16 = mybir.dt.bfloat16
    f32 = mybir.dt.float32
    AF = mybir.ActivationFunctionType
    Sig, Tanh = AF.Sigmoid, AF.Tanh

    # column counts (mirror Pack layout; data-independent)
    n_wih = 29 * 64
    n_whh = 23 * 64
    conv_cols = {"f12h": 32, "f12c": 32, "f23h": 32, "f23c": 32, "f34h": 32,
                 "f34c": 32, "b43h": 64, "b43c": 64, "b32h": 64, "b32c": 64,
                 "b21h": 64, "b21c": 64}
    conv_order = list(conv_cols.keys())
    conv_off = {}
    c = 0
    for k in conv_order:
        conv_off[k] = c
        c += conv_cols[k]
    n_conv = c
    ones_off = {8: 0, 16: 128, 32: 256, 64: 384}

    wih_names = []
    for d in ("f", "b"):
        for lvl in (1, 2, 3, 4):
            first = (d == "f" and lvl == 1) or (d == "b" and lvl == 4)
            last = d == "b" and lvl == 1
            gates = ("i", "g", "o") if first else (("i", "f", "g") if last else GATES)
            for gt in gates:
                wih_names.append(f"{d}{lvl}_{gt}")
    wih_off = {k: 64 * i for i, k in enumerate(wih_names)}
    bias_off = {k: i for i, k in enumerate(wih_names)}  # same ordering

    whh_names = [f"f{l}_{g}" for l in (2, 3, 4) for g in GATES]
    whh_names += [f"b{l}_{g}" for l in (3, 2) for g in GATES]
    whh_names += [f"b1_{g}" for g in ("i", "f", "g")]
    whh_off = {k: 64 * i for i, k in enumerate(whh_names)}
    whh_K = {}
    for k in whh_names:
        d, lvl = k[0], int(k[1])
        if d == "f":
            whh_K[k] = {2: 32, 3: 32, 4: 32}[lvl]
        else:
            whh_K[k] = 64

    nc = bacc.Bacc("TRN2", target_bir_lowering=False, debug=False)
    xT = nc.dram_tensor("xT", (128, n_samples), bf16, kind="ExternalInput")
    outT = nc.dram_tensor("outT", (128, n_samples), bf16, kind="ExternalOutput")
    wih_d = nc.dram_tensor("wih", (128, n_wih), bf16, kind="ExternalInput")
    whh_d = nc.dram_tensor("whh", (128, n_whh), bf16, kind="ExternalInput")
    wconv_d = nc.dram_tensor("wconv", (128, n_conv), bf16, kind="ExternalInput")
    wones_d = nc.dram_tensor("wones", (128, 512), bf16, kind="ExternalInput")
    bias_d = nc.dram_tensor("biasm", (128, 29), f32, kind="ExternalInput")

    npairs = n_samples // (2 * NT)
    xT_ap, outT_ap = xT.ap(), outT.ap()

    with TileContext(nc) as tc:
        with (
            tc.tile_pool(name="wpool", bufs=1) as wpool,
            tc.tile_pool(name="xpool", bufs=POOLCFG["xpool"]) as xpool,
            tc.tile_pool(name="spool", bufs=POOLCFG["spool"]) as spool,
            tc.tile_pool(name="opool", bufs=2) as opool,
            tc.tile_pool(name="pgf", bufs=POOLCFG["pgf"], space="PSUM") as pgf,
            tc.tile_pool(name="pgb", bufs=POOLCFG["pgb"], space="PSUM") as pgb,
            tc.tile_pool(name="pc", bufs=POOLCFG["pc"], space="PSUM") as pc,
            tc.tile_pool(name="pd", bufs=POOLCFG["pd"], space="PSUM") as pd,
        ):
            wih_sb = wpool.tile([128, n_wih], bf16)
            nc.sync.dma_start(out=wih_sb[:], in_=wih_d.ap()[:, :])
            whh_sb = wpool.tile([128, n_whh], bf16)
            nc.sync.dma_start(out=whh_sb[:], in_=whh_d.ap()[:, :])
            wconv_sb = wpool.tile([128, n_conv], bf16)
            nc.sync.dma_start(out=wconv_sb[:], in_=wconv_d.ap()[:, :])
            wones_sb = wpool.tile([128, 512], bf16)
            nc.sync.dma_start(out=wones_sb[:], in_=wones_d.ap()[:, :])
            bias_sb = wpool.tile([128, 29], f32)
            nc.sync.dma_start(out=bias_sb[:], in_=bias_d.ap()[:, :])

            def exp4(out_ap, in_ap):
                nc.vector._custom_dve(EXP4, out=out_ap, in0=in_ap,
                                      s0=EXP_C1, s1=EXP_C2, imm2=EXP_C3)

            def tanh_mul(c_ap, s_ap, tag, out=None):
                if out is None:
                    out = spool.tile([128, NT], bf16, tag=tag, name=tag)[:]
                nc.vector._custom_dve(TMUL, out=out, in0=c_ap, in1=s_ap,
                                      s0=TANH_A, s1=TANH_B, imm2=TANH_C)
                return out

            def recip(out_ap, in_ap):
                nc.vector._custom_dve(RECIPROCAL_APPROX_FAST, out=out_ap,
                                      in0=in_ap, s0=RC["s0"], s1=RC["s1"],
                                      imm2=RC["imm2"])

            def gates_mm(d, lvl, gates, XA, XB, hbA=None, hbB=None):
                """Returns dict gate -> psum tile [128, NT] (A rows 0:64,
                B rows 64:128), biases already folded in at the ACT step."""
                ps = {}
                pool = pgf if d == "f" else pgb
                for gt in gates:
                    name = f"{d}{lvl}_{gt}"
                    p = pool.tile([128, NT], f32, tag="gates", name="gates")
                    w = wih_sb[:, wih_off[name]:wih_off[name] + 64]
                    has_hh = hbA is not None and name in whh_off
                    if has_hh:
                        K = whh_K[name]
                        o = whh_off[name]
                        baseA = hbA.base_partition()
                        baseB = hbB.base_partition()
                        wA = whh_sb[baseA:baseA + K, o:o + 64]
                        wB = whh_sb[baseB:baseB + K, o:o + 64]
                    # NB: each half's accumulation group must close before the
                    # other half's opens — PSUM zero-regions are bank-granular.
                    nc.tensor.matmul(p[0:64, :], w, XA[:],
                                     start=True, stop=not has_hh)
                    if has_hh:
                        nc.tensor.matmul(p[0:64, :], wA, hbA,
                                         start=False, stop=True)
                    nc.tensor.matmul(p[64:128, :], w, XB[:],
                                     start=True, stop=not has_hh)
                    if has_hh:
                        nc.tensor.matmul(p[64:128, :], wB, hbB,
                                         start=False, stop=True)
                    ps[gt] = p
                return ps

            def act(func, ps_tile, d, lvl, gt, rows=None):
                name = f"{d}{lvl}_{gt}"
                o = spool.tile([128, NT], bf16, tag=f"a_{gt}")
                b = bias_sb[:, bias_off[name]:bias_off[name] + 1]
                nc.scalar.activation(out=o[:], in_=ps_tile[:], func=func,
                                     bias=b, scale=1.0)
                return o

            def tanh_sbuf(t_in, tag):
                o = spool.tile([128, NT], bf16, tag=tag)
                nc.scalar.activation(out=o[:], in_=t_in[:], func=Tanh)
                return o

            def tt(op, a, b, tag=None, out=None):
                if out is None:
                    out = spool.tile([128, NT], bf16, tag=tag, name=tag)[:]
                if op == "mul":
                    nc.vector.tensor_mul(out, a, b)
                else:
                    nc.vector.tensor_add(out, a, b)
                return out

            def trans_fwd(stage, h_pair, c_pair, width):
                oh, oc = conv_off[stage + "h"], conv_off[stage + "c"]
                e_ps = pc.tile([128, NT], f32, tag="eps")
                nc.tensor.matmul(e_ps[0:32, :], wconv_sb[0:64, oh:oh + 32],
                                 h_pair[0:64, :], start=True, stop=True,
                                 tile_position=(0, 0))
                nc.tensor.matmul(e_ps[32:64, :], wconv_sb[0:64, oc:oc + 32],
                                 c_pair[0:64, :], start=True, stop=True,
                                 tile_position=(0, 32))
                nc.tensor.matmul(e_ps[64:96, :], wconv_sb[64:128, oh:oh + 32],
                                 h_pair[64:128, :], start=True, stop=True,
                                 tile_position=(64, 64))
                nc.tensor.matmul(e_ps[96:128, :], wconv_sb[64:128, oc:oc + 32],
                                 c_pair[64:128, :], start=True, stop=True,
                                 tile_position=(64, 96))
                e = spool.tile([128, NT], bf16, tag="e")
                exp4(e[:], e_ps[:])
                d_ps = pd.tile([128, NT], f32, tag="dps")
                oo = ones_off[width]
                nc.tensor.matmul(d_ps[:], wones_sb[:, oo:oo + 128], e[:],
                                 start=True, stop=True)
                nb = spool.tile([128, NT], bf16, tag="nb_" + stage)
                nc.vector._custom_dve(RMUL, out=nb[:], in0=d_ps[:], in1=e[:],
                                      s0=RC["s0"], s1=RC["s1"])
                return nb

            def trans_bwd(stage, h_pair, c_pair, width):
                """Per-tile trans for the backward chain.
                Returns (nbA, nbB): A rows = [cb 0:64 | hb 64:128],
                B rows = [hb 0:64 | cb 64:128]."""
                oh, oc = conv_off[stage + "h"], conv_off[stage + "c"]
                oo = ones_off[width]
                nbs = []
                for which in ("A", "B"):
                    e_ps = pc.tile([128, NT], f32, tag="eps")
                    if which == "A":
                        rh, rc = h_pair[0:64, :], c_pair[0:64, :]
                        wslice = wconv_sb[0:64, :]
                        c_rows, h_rows = (0, 64), (64, 128)
                    else:
                        rh, rc = h_pair[64:128, :], c_pair[64:128, :]
                        wslice = wconv_sb[64:128, :]
                        c_rows, h_rows = (64, 128), (0, 64)
                    nc.tensor.matmul(e_ps[c_rows[0]:c_rows[1], :],
                                     wslice[:, oc:oc + 64], rc,
                                     start=True, stop=True)
                    nc.tensor.matmul(e_ps[h_rows[0]:h_rows[1], :],
                                     wslice[:, oh:oh + 64], rh,
                                     start=True, stop=True)
                    e = spool.tile([128, NT], bf16, tag="e")
                    exp4(e[:], e_ps[:])
                    d_ps = pd.tile([128, NT], f32, tag="dps")
                    nc.tensor.matmul(d_ps[:], wones_sb[:, oo:oo + 128], e[:],
                                     start=True, stop=True)
                    nb = spool.tile([128, NT], bf16, tag=f"nb{which}_" + stage)
                    nc.vector._custom_dve(RMUL, out=nb[:], in0=d_ps[:],
                                          in1=e[:], s0=RC["s0"], s1=RC["s1"])
                    nbs.append(nb)
                return nbs

            def fwd_chain(XA, XB, cA, cB):
                # ---------------- forward ----------------
                G = gates_mm("f", 1, ("i", "g", "o"), XA, XB)
                sI = act(Sig, G["i"], "f", 1, "i")
                sO = act(Sig, G["o"], "f", 1, "o")
                tG = act(Tanh, G["g"], "f", 1, "g")
                c1 = tt("mul", sI[:], tG[:], tag="cst")
                h1 = tanh_mul(c1, sO[:], "h")
                yield
                nb12 = trans_fwd("f12", h1, c1, 32)

                # dup cb for level 2 (2 steps x 32)
                cbd = spool.tile([128, NT], bf16, tag="cbd")
                nc.sync.dma_start(out=cbd[0:32, :], in_=nb12[32:64, :])
                nc.gpsimd.dma_start(out=cbd[32:64, :], in_=nb12[32:64, :])
                nc.sync.dma_start(out=cbd[64:96, :], in_=nb12[96:128, :])
                nc.gpsimd.dma_start(out=cbd[96:128, :], in_=nb12[96:128, :])
                yield

                G = gates_mm("f", 2, GATES, XA, XB,
                             nb12[0:32, :], nb12[64:96, :])
                sI = act(Sig, G["i"], "f", 2, "i")
                sF = act(Sig, G["f"], "f", 2, "f")
                sO = act(Sig, G["o"], "f", 2, "o")
                tG = act(Tanh, G["g"], "f", 2, "g")
                t1 = tt("mul", sI[:], tG[:], tag="t1")
                t2 = tt("mul", sF[:], cbd[:], tag="t2")
                c2 = tt("add", t1, t2, tag="cst")
                h2 = tanh_mul(c2, sO[:], "h")
                yield
                nb23 = trans_fwd("f23", h2, c2, 16)

                # dup cb for level 3: steps s use block s//2 (16 wide)
                cbd = spool.tile([128, NT], bf16, tag="cbd")
                for base, src in ((0, 32), (64, 96)):
                    for s in range(4):
                        blk = src + 16 * (s // 2)
                        eng = nc.sync if s % 2 else nc.gpsimd
                        eng.dma_start(
                            out=cbd[base + 16 * s:base + 16 * s + 16, :],
                            in_=nb23[blk:blk + 16, :])
                yield

                G = gates_mm("f", 3, GATES, XA, XB,
                             nb23[0:32, :], nb23[64:96, :])
                sI = act(Sig, G["i"], "f", 3, "i")
                sF = act(Sig, G["f"], "f", 3, "f")
                sO = act(Sig, G["o"], "f", 3, "o")
                tG = act(Tanh, G["g"], "f", 3, "g")
                t1 = tt("mul", sI[:], tG[:], tag="t1")
                t2 = tt("mul", sF[:], cbd[:], tag="t2")
                c3 = tt("add", t1, t2, tag="cst")
                h3 = tanh_mul(c3, sO[:], "h")
                yield
                nb34 = trans_fwd("f34", h3, c3, 8)

                # dup cb for level 4: steps s use block s//2 (8 wide)
                cbd = spool.tile([128, NT], bf16, tag="cbd")
                for base, src in ((0, 32), (64, 96)):
                    for s in range(8):
                        blk = src + 8 * (s // 2)
                        eng = nc.sync if s % 2 else nc.gpsimd
                        eng.dma_start(
                            out=cbd[base + 8 * s:base + 8 * s + 8, :],
                            in_=nb34[blk:blk + 8, :])
                yield

                G = gates_mm("f", 4, GATES, XA, XB,
                             nb34[0:32, :], nb34[64:96, :])
                sI = act(Sig, G["i"], "f", 4, "i")
                sF = act(Sig, G["f"], "f", 4, "f")
                sO = act(Sig, G["o"], "f", 4, "o")
                tG = act(Tanh, G["g"], "f", 4, "g")
                t1 = tt("mul", sI[:], tG[:], tag="t1")
                t2 = tt("mul", sF[:], cbd[:], tag="t2")
                c4 = tt("add", t1, t2, tag="cst")
                tC = tanh_sbuf(c4, "tC")
                outh = opool.tile([128, NT], bf16, tag="outh")
                tt("mul", sO[:], tC[:], out=outh[:])
                nc.sync.dma_start(out=outT_ap[0:64, cA], in_=outh[0:64, :])
                nc.sync.dma_start(out=outT_ap[0:64, cB], in_=outh[64:128, :])

            def bwd_chain(XA, XB, cA, cB):
                # ---------------- backward ----------------
                G = gates_mm("b", 4, ("i", "g", "o"), XA, XB)
                sI = act(Sig, G["i"], "b", 4, "i")
                sO = act(Sig, G["o"], "b", 4, "o")
                tG = act(Tanh, G["g"], "b", 4, "g")
                c4b = tt("mul", sI[:], tG[:], tag="cstb")
                h4b = tanh_mul(c4b, sO[:], "hb")
                yield
                nbA, nbB = trans_bwd("b43", h4b, c4b, 16)
                yield

                G = gates_mm("b", 3, GATES, XA, XB,
                             nbA[64:128, :], nbB[0:64, :])
                sI = act(Sig, G["i"], "b", 3, "i")
                sF = act(Sig, G["f"], "b", 3, "f")
                sO = act(Sig, G["o"], "b", 3, "o")
                tG = act(Tanh, G["g"], "b", 3, "g")
                t1 = tt("mul", sI[:], tG[:], tag="t1b")
                t2b = spool.tile([128, NT], bf16, tag="t2b", name="t2b")
                nc.vector.tensor_mul(t2b[0:64, :], sF[0:64, :],
                                     nbA[0:64, :])
                nc.vector.tensor_mul(t2b[64:128, :], sF[64:128, :],
                                     nbB[64:128, :])
                c3b = tt("add", t1, t2b[:], tag="cstb")
                h3b = tanh_mul(c3b, sO[:], "hb")
                yield
                nbA, nbB = trans_bwd("b32", h3b, c3b, 32)
                yield

                G = gates_mm("b", 2, GATES, XA, XB,
                             nbA[64:128, :], nbB[0:64, :])
                sI = act(Sig, G["i"], "b", 2, "i")
                sF = act(Sig, G["f"], "b", 2, "f")
                sO = act(Sig, G["o"], "b", 2, "o")
                tG = act(Tanh, G["g"], "b", 2, "g")
                t1 = tt("mul", sI[:], tG[:], tag="t1b")
                t2b = spool.tile([128, NT], bf16, tag="t2b", name="t2b")
                nc.vector.tensor_mul(t2b[0:64, :], sF[0:64, :],
                                     nbA[0:64, :])
                nc.vector.tensor_mul(t2b[64:128, :], sF[64:128, :],
                                     nbB[64:128, :])
                c2b = tt("add", t1, t2b[:], tag="cstb")
                h2b = tanh_mul(c2b, sO[:], "hb")
                yield
                nbA, nbB = trans_bwd("b21", h2b, c2b, 64)
                yield

                G = gates_mm("b", 1, ("i", "f", "g"), XA, XB,
                             nbA[64:128, :], nbB[0:64, :])
                sI = act(Sig, G["i"], "b", 1, "i")
                sF = act(Sig, G["f"], "b", 1, "f")
                tG = act(Tanh, G["g"], "b", 1, "g")
                t1 = tt("mul", sI[:], tG[:], tag="t1b")
                t2b = spool.tile([128, NT], bf16, tag="t2b", name="t2b")
                nc.vector.tensor_mul(t2b[0:64, :], sF[0:64, :],
                                     nbA[0:64, :])
                nc.vector.tensor_mul(t2b[64:128, :], sF[64:128, :],
                                     nbB[64:128, :])
                outc = opool.tile([128, NT], bf16, tag="outc")
                tt("add", t1, t2b[:], out=outc[:])
                nc.sync.dma_start(out=outT_ap[64:128, cA], in_=outc[0:64, :])
                nc.sync.dma_start(out=outT_ap[64:128, cB],
                                  in_=outc[64:128, :])

            import itertools

            def pair_chains(p):
                cA = slice(2 * p * NT, 2 * p * NT + NT)
                cB = slice(2 * p * NT + NT, 2 * p * NT + 2 * NT)
                XA = xpool.tile([128, NT], bf16, tag="XA", name="XA")
                nc.sync.dma_start(out=XA[:], in_=xT_ap[:, cA])
                XB = xpool.tile([128, NT], bf16, tag="XB", name="XB")
                nc.sync.dma_start(out=XB[:], in_=xT_ap[:, cB])
                return (fwd_chain(XA, XB, cA, cB), bwd_chain(XA, XB, cA, cB))

            GROUP = POOLCFG.get("group", 1)
            for p0 in range(0, npairs, GROUP):
                chains = []
                for p in range(p0, min(p0 + GROUP, npairs)):
                    chains.extend(pair_chains(p))
                for _ in itertools.zip_longest(*chains):
                    pass

    nc.compile()
    _BUILD_CACHE[key] = nc
    return nc


def _bf16(a):
    return np.ascontiguousarray(a).astype(ml_dtypes.bfloat16)


def kernel(**inputs):
    from concourse.bass_utils import run_bass_kernel_spmd

    pack = Pack(inputs)
    spec = np.asarray(inputs["spec"], np.float32)
    b_total = spec.shape[0]
    s_core = b_total // NCORES
    xT = np.ascontiguousarray(spec.T)          # [128, B]

    nc = _build(s_core)

    wih = _bf16(pack.wih_mat)
    whh = _bf16(pack.whh_mat)
    wconv = _bf16(pack.wconv_mat)
    wones = _bf16(pack.wones_mat)
    biasm = np.ascontiguousarray(pack.bias_mat, dtype=np.float32)

    in_maps = []
    for k in range(NCORES):
        in_maps.append({
            "xT": _bf16(xT[:, k * s_core:(k + 1) * s_core]),
            "wih": wih, "whh": whh, "wconv": wconv, "wones": wones,
            "biasm": biasm,
        })
    res = run_bass_kernel_spmd(nc, in_maps, core_ids=list(range(NCORES)))

    out = np.empty((b_total, 128), np.float32)
    for k in range(NCORES):
        o = np.asarray(res.results[k]["outT"]).astype(np.float32)  # [128, S]
        out[k * s_core:(k + 1) * s_core, :] = o.T
    return out


if __name__ == "__main__":
    pass



# revision 9
# speedup vs baseline: 2.5767x; 2.5767x over previous
"""Trainium2 Bass kernel for nn_DSGSF (batched bidirectional multi-scale LSTM).

Transfer-optimized design. The axon tunnel moves ~65MB/s up / ~47MB/s down and
serializes all traffic, while the device itself needs only ~5ms — so the
kernel minimizes tunnel bytes and RPC count:

  - input spec is shipped as a 12-bit quantization: an int8 "hi" plane plus a
    packed-nibble "lo" plane (1.5 B/sample-feature instead of 2 B for bf16;
    also ~4x less quantization noise than bf16's 8-bit mantissa);
  - the quantization scale is folded into the host-packed W_ih matrices, and
    gates are computed as (W')*hi + (W')*(lo/16) with two bf16 matmuls
    sharing one stationary weight;
  - output is shipped as int8 with per-row per-tile scales computed on
    device (absmax reduce -> reciprocal -> scale+convert), plus a tiny
    [128, 2*npairs] f32 scale tensor;
  - everything runs on ONE core (device compute is ~1000x faster than the
    tunnel; splitting across cores only multiplies per-transfer overhead);
  - a cached jax.jit around the bass_exec primitive avoids per-call
    retrace/relower, and output operand buffers are created on-device
    (jnp.zeros) instead of being shipped from host.

On-device math is unchanged from the baseline: transposed layout
(features/gates on partitions, samples on the free dim), block-diagonal
lhsT matmuls per gate covering all steps of a level, conv1d band matrices
with the exp 1/4 prescale folded in, block-ones matmuls for softmax
denominators, sigmoid/tanh on ACT, exp as a fused cubic-poly DVE op,
reciprocal via RECIPROCAL_APPROX_FAST.
"""

import threading

import numpy as np
import ml_dtypes

B, D, H = 131072, 128, 64
NT = 512                 # samples per tile (one PSUM bank)
QBITS_MAX = 2032         # 12-bit quant: q in [-2032, 2032], hi=q>>4, lo=q&15

GATES = ("i", "f", "g", "o")

# ---------------------------------------------------------------------------
# exp4 polynomial: exp(y) = p(y/4)^4 with p(u) = 1 + c1 u + c2 u^2 + c3 u^3
# fitted for relative error of p(u)^4 vs e^(4u) on |u| <= EXP_FIT_RANGE.
EXP_FIT_RANGE = 0.85


def _fit_exp_poly():
    u = np.linspace(-EXP_FIT_RANGE, EXP_FIT_RANGE, 4001)
    A = np.stack([u, u * u, u**3], axis=1) / np.exp(u)[:, None]
    b = (np.exp(u) - 1.0) / np.exp(u)
    c, *_ = np.linalg.lstsq(A, b, rcond=None)
    return c


EXP_C1, EXP_C2, EXP_C3 = (float(v) for v in _fit_exp_poly())


def _fit_tanh5():
    u = np.linspace(-2.05, 2.05, 4001)
    A = np.stack([u, u**3, u**5], axis=1)
    c, *_ = np.linalg.lstsq(A, np.tanh(u), rcond=None)
    return c


TANH_A, TANH_B, TANH_C = (float(v) for v in _fit_tanh5())


def tanh5_np(u):
    return u * (TANH_A + TANH_B * u * u + TANH_C * u**4)


def exp4_np(y):
    u = y.astype(np.float64)
    p = 1.0 + u * (EXP_C1 + u * (EXP_C2 + EXP_C3 * u))
    return (p * p) ** 2


def recip_np(x):
    x = x.astype(np.float32)
    nx = (~x.view(np.int32)).view(np.float32)
    y0 = nx * np.float32(-0.23549792)
    y1 = y0 * (np.float32(2.0017324) - x * y0)
    return y1 * (np.float32(2.0) - x * y1)


def _sigmoid(x):
    return 1.0 / (1.0 + np.exp(-x))


# ---------------------------------------------------------------------------
# Host-side weight packing.
LEVEL_STEPS = {1: 1, 2: 2, 3: 4, 4: 8}


def _ih_lhsT(w_ih, level, gate):
    """Block-diagonal lhsT [128, 64] computing `gate` preacts for all steps."""
    Sl = LEVEL_STEPS[level]
    dh, di = 64 // Sl, 128 // Sl
    gi = GATES.index(gate)
    wg = w_ih[gi * dh:(gi + 1) * dh, :]
    out = np.zeros((128, 64), np.float32)
    for s in range(Sl):
        out[s * di:(s + 1) * di, s * dh:(s + 1) * dh] = wg.T
    return out


def _hh_lhsT(w_hh, level, gate, direction):
    """lhsT [K, 64] mapping hb rows onto gate preacts for all steps."""
    Sl = LEVEL_STEPS[level]
    dh = 64 // Sl
    dprev = w_hh.shape[1]
    gi = GATES.index(gate)
    wg = w_hh[gi * dh:(gi + 1) * dh, :]
    if direction == "f":
        nblk = {2: 1, 3: 2, 4: 4}[level]
        K = nblk * dprev
        out = np.zeros((K, 64), np.float32)
        for s in range(Sl):
            blk = s * nblk // Sl
            out[blk * dprev:(blk + 1) * dprev, s * dh:(s + 1) * dh] = wg.T
    else:
        K = Sl * dprev
        out = np.zeros((K, 64), np.float32)
        for s in range(Sl):
            out[s * dprev:(s + 1) * dprev, s * dh:(s + 1) * dh] = wg.T
    return out


def _conv_lhsT(w3, win, nblk, stride, scale):
    """Band matrix [nblk*win, nblk*wout] for blockwise conv1d(k=3,pad=1)."""
    w = np.asarray(w3, np.float64).reshape(3) * scale
    wout = win // stride
    blk = np.zeros((win, wout), np.float64)
    for j in range(wout):
        for t in range(3):
            k = stride * j - 1 + t
            if 0 <= k < win:
                blk[k, j] += w[t]
    out = np.zeros((nblk * win, nblk * wout), np.float32)
    for q in range(nblk):
        out[q * win:(q + 1) * win, q * wout:(q + 1) * wout] = blk
    return out


def _ones_block(width):
    out = np.zeros((128, 128), np.float32)
    for q in range(128 // width):
        out[q * width:(q + 1) * width, q * width:(q + 1) * width] = 1.0
    return out


def _bias_pair(b, level, gate):
    """[128] bias rows for a pair-packed gate psum tile (same 64 twice)."""
    Sl = LEVEL_STEPS[level]
    dh = 64 // Sl
    gi = GATES.index(gate)
    bg = np.asarray(b, np.float32)[gi * dh:(gi + 1) * dh]
    one = np.tile(bg, Sl)
    return np.concatenate([one, one])


class Pack:
    """All host-packed constant matrices + column offset maps.

    xscale is folded into the wih blocks: device rhs is (hi + lo/16) and
    x_hat = xscale * (hi + lo/16), so wih' = wih * xscale.
    """

    def __init__(self, inp, xscale=1.0):
        g = lambda n: np.asarray(inp[n], np.float32)

        self.wih = {}
        order = []
        for d in ("f", "b"):
            for lvl in (1, 2, 3, 4):
                w = g(f"{d}w_ih{lvl}")
                first = (d == "f" and lvl == 1) or (d == "b" and lvl == 4)
                last = d == "b" and lvl == 1
                gates = ("i", "g", "o") if first else (("i", "f", "g") if last else GATES)
                for gt in gates:
                    self.wih[f"{d}{lvl}_{gt}"] = _ih_lhsT(w, lvl, gt) * xscale
                    order.append(f"{d}{lvl}_{gt}")
        self.wih_order = order
        self.wih_mat = np.concatenate([self.wih[k] for k in order], axis=1)

        # hh lhsT blocks, replicated at partition offset 64 for B-tile matmuls
        self.whh = {}
        horder = []
        for lvl in (2, 3, 4):
            w = g(f"fw_hh{lvl}")
            for gt in GATES:
                self.whh[f"f{lvl}_{gt}"] = _hh_lhsT(w, lvl, gt, "f")
                horder.append(f"f{lvl}_{gt}")
        for lvl in (3, 2, 1):
            w = g(f"bw_hh{lvl}")
            gates = ("i", "f", "g") if lvl == 1 else GATES
            for gt in gates:
                self.whh[f"b{lvl}_{gt}"] = _hh_lhsT(w, lvl, gt, "b")
                horder.append(f"b{lvl}_{gt}")
        self.whh_order = horder
        self.whh_K = {k: self.whh[k].shape[0] for k in horder}
        mats = []
        for k in horder:
            m = np.zeros((128, 64), np.float32)
            m[: self.whh_K[k], :] = self.whh[k]
            m[64:64 + self.whh_K[k], :] = self.whh[k]
            mats.append(m)
        self.whh_mat = np.concatenate(mats, axis=1)

        sc = 0.25
        self.wconv = {
            "f12h": _conv_lhsT(g("ft12h"), 64, 1, 2, sc),
            "f12c": _conv_lhsT(g("ft12c"), 64, 1, 2, sc),
            "f23h": _conv_lhsT(g("ft23h"), 32, 2, 2, sc),
            "f23c": _conv_lhsT(g("ft23c"), 32, 2, 2, sc),
            "f34h": _conv_lhsT(g("ft34h"), 16, 4, 2, sc),
            # NB: reference reuses ft34h for the c path (original model bug)
            "f34c": _conv_lhsT(g("ft34h"), 16, 4, 2, sc),
            "b43h": _conv_lhsT(g("bt43h"), 16, 4, 1, sc),
            "b43c": _conv_lhsT(g("bt43c"), 16, 4, 1, sc),
            "b32h": _conv_lhsT(g("bt32h"), 32, 2, 1, sc),
            "b32c": _conv_lhsT(g("bt32c"), 32, 2, 1, sc),
            "b21h": _conv_lhsT(g("bt21h"), 64, 1, 1, sc),
            "b21c": _conv_lhsT(g("bt21c"), 64, 1, 1, sc),
        }
        self.conv_order = list(self.wconv.keys())
        cmats = []
        for k in self.conv_order:
            c = self.wconv[k]
            m = np.zeros((128, c.shape[1]), np.float32)
            m[:64] = c
            m[64:] = c
            cmats.append(m)
        self.wconv_mat = np.concatenate(cmats, axis=1)

        self.wones = {w: _ones_block(w) for w in (8, 16, 32, 64)}
        self.ones_order = [8, 16, 32, 64]
        self.wones_mat = np.concatenate(
            [self.wones[w] for w in self.ones_order], axis=1)

        self.bias = {}
        border = []
        for d in ("f", "b"):
            for lvl in (1, 2, 3, 4):
                b = g(f"{d}b{lvl}")
                first = (d == "f" and lvl == 1) or (d == "b" and lvl == 4)
                last = d == "b" and lvl == 1
                gates = ("i", "g", "o") if first else (("i", "f", "g") if last else GATES)
                for gt in gates:
                    self.bias[f"{d}{lvl}_{gt}"] = _bias_pair(b, lvl, gt)
                    border.append(f"{d}{lvl}_{gt}")
        self.bias_order = border
        self.bias_mat = np.stack([self.bias[k] for k in border], axis=1)

        self.wih_off = {k: 64 * i for i, k in enumerate(order)}
        self.whh_off = {k: 64 * i for i, k in enumerate(horder)}
        off = {}
        c = 0
        for k in self.conv_order:
            off[k] = c
            c += self.wconv[k].shape[1]
        self.conv_off = off
        self.ones_off = {w: 128 * i for i, w in enumerate(self.ones_order)}
        self.bias_off = {k: i for i, k in enumerate(border)}

    def wall(self):
        """Single merged bf16 weight matrix [128, ncols]:
        wih | whh | wconv | wones | bias-as-bf16."""
        bias_bf = self.bias_mat.astype(ml_dtypes.bfloat16).astype(np.float32)
        mats = [self.wih_mat, self.whh_mat, self.wconv_mat, self.wones_mat,
                bias_bf]
        return np.concatenate(mats, axis=1).astype(ml_dtypes.bfloat16)


# column layout of the merged weight tensor (data-independent)
N_WIH = 29 * 64
N_WHH = 23 * 64
CONV_COLS = {"f12h": 32, "f12c": 32, "f23h": 32, "f23c": 32, "f34h": 32,
             "f34c": 32, "b43h": 64, "b43c": 64, "b32h": 64, "b32c": 64,
             "b21h": 64, "b21c": 64}
N_CONV = sum(CONV_COLS.values())
N_ONES = 512
N_BIAS = 29
N_WALL = N_WIH + N_WHH + N_CONV + N_ONES + N_BIAS


# ---------------------------------------------------------------------------
# Numpy mirror of the device program (for validation; PO=0, one tile).

def mirror_forward(pack: Pack, specT, exact=False):
    """specT: [128, n] float32. Returns outT [128, n]."""
    X = specT.astype(np.float32)
    myexp = (lambda y: np.exp(4.0 * y)) if exact else exp4_np
    myrecip = (lambda x: 1.0 / x) if exact else recip_np

    def mm(lhsT, rhs):
        return lhsT.T.astype(np.float32) @ rhs.astype(np.float32)

    def gates_ih(d, lvl, rhs):
        return {gt: mm(pack.wih[f"{d}{lvl}_{gt}"], rhs)
                for gt in GATES
                if f"{d}{lvl}_{gt}" in pack.wih}

    def add_hh(G, d, lvl, hb):
        for gt in list(G):
            G[gt] = G[gt] + mm(pack.whh[f"{d}{lvl}_{gt}"], hb)

    def add_bias(G, d, lvl):
        for gt in list(G):
            G[gt] = G[gt] + pack.bias[f"{d}{lvl}_{gt}"][:64, None]

    def trans(name_h, name_c, h, c, widths):
        eh = myexp(mm(pack.wconv[name_h], h))
        ec = myexp(mm(pack.wconv[name_c], c))
        e = np.concatenate([eh, ec], axis=0)
        ones = pack.wones[widths][: e.shape[0], : e.shape[0]]
        d_bc = mm(ones, e)
        inv = myrecip(d_bc.astype(np.float32))
        nb = e * inv
        half = eh.shape[0]
        return nb[:half], nb[half:]

    G = gates_ih("f", 1, X); add_bias(G, "f", 1)
    sI, sO, tG = _sigmoid(G["i"]), _sigmoid(G["o"]), np.tanh(G["g"])
    c1 = sI * tG
    h1 = sO * np.tanh(c1)
    hb1, cb1 = trans("f12h", "f12c", h1, c1, 32)

    G = gates_ih("f", 2, X); add_hh(G, "f", 2, hb1); add_bias(G, "f", 2)
    cb1d = np.concatenate([cb1, cb1], axis=0)
    t1 = _sigmoid(G["i"]) * np.tanh(G["g"])
    c2 = _sigmoid(G["f"]) * cb1d + t1
    h2 = _sigmoid(G["o"]) * np.tanh(c2)
    hb2, cb2 = trans("f23h", "f23c", h2, c2, 16)

    G = gates_ih("f", 3, X); add_hh(G, "f", 3, hb2); add_bias(G, "f", 3)
    cb2d = np.concatenate([cb2[0:16], cb2[0:16], cb2[16:32], cb2[16:32]], axis=0)
    t1 = _sigmoid(G["i"]) * np.tanh(G["g"])
    c3 = _sigmoid(G["f"]) * cb2d + t1
    h3 = _sigmoid(G["o"]) * np.tanh(c3)
    hb3, cb3 = trans("f34h", "f34c", h3, c3, 8)

    G = gates_ih("f", 4, X); add_hh(G, "f", 4, hb3); add_bias(G, "f", 4)
    cb3d = np.concatenate(
        [cb3[8 * (s // 2):8 * (s // 2) + 8] for s in range(8)], axis=0)
    t1 = _sigmoid(G["i"]) * np.tanh(G["g"])
    c4 = _sigmoid(G["f"]) * cb3d + t1
    h4 = _sigmoid(G["o"]) * np.tanh(c4)

    G = gates_ih("b", 4, X); add_bias(G, "b", 4)
    c4b = _sigmoid(G["i"]) * np.tanh(G["g"])
    h4b = _sigmoid(G["o"]) * np.tanh(c4b)
    hb4, cb4 = trans("b43h", "b43c", h4b, c4b, 16)

    G = gates_ih("b", 3, X); add_hh(G, "b", 3, hb4); add_bias(G, "b", 3)
    t1 = _sigmoid(G["i"]) * np.tanh(G["g"])
    c3b = _sigmoid(G["f"]) * cb4 + t1
    h3b = _sigmoid(G["o"]) * np.tanh(c3b)
    hb3b, cb3b = trans("b32h", "b32c", h3b, c3b, 32)

    G = gates_ih("b", 2, X); add_hh(G, "b", 2, hb3b); add_bias(G, "b", 2)
    t1 = _sigmoid(G["i"]) * np.tanh(G["g"])
    c2b = _sigmoid(G["f"]) * cb3b + t1
    h2b = _sigmoid(G["o"]) * np.tanh(c2b)
    hb2b, cb2b = trans("b21h", "b21c", h2b, c2b, 64)

    G = gates_ih("b", 1, X); add_hh(G, "b", 1, hb2b); add_bias(G, "b", 1)
    c1b = _sigmoid(G["f"]) * cb2b + _sigmoid(G["i"]) * np.tanh(G["g"])

    return np.concatenate([h4, c1b], axis=0)


# ---------------------------------------------------------------------------
# Custom DVE ops.

def _register_op(op):
    import re
    import concourse.dve_ops as dve_ops

    dve_ops.OPS.append(op)
    dve_ops._SUB_OPCODE_FOR_NAME[op.name] = (
        dve_ops._CUSTOM_DVE_ROW_BASE + len(dve_ops.OPS) - 1)
    dve_ops.CUSTOM_DVE_SPECS[op.name] = op.spec
    for ver in ("v3",):
        try:
            op.compile(ver)
        except ValueError as e:
            m = re.search(rf"\({ver}: ([0-9a-f]+)", str(e))
            if not m:
                raise
            op.uops_sha[ver] = m.group(1)
            op.compile(ver)


def _register_exp4():
    import concourse.dve_ops as dve_ops
    from concourse.dve_ops import DveOp
    from concourse.dve_spec import Spec, Src0, C0, C1, C2, One

    for op in dve_ops.OPS:
        if op.name == "EXP4_ANT":
            return op
    u = Src0
    inner = C1 + C2 * u
    inner2 = C0 + u * inner
    p = One + u * inner2
    sq = p * p
    spec = Spec(
        body=sq * sq,
        reference=lambda in0, in1, s0, s1, imm2:
            (1.0 + in0 * (s0 + in0 * (s1 + imm2 * in0))) ** 4,
    )
    op = DveOp("EXP4_ANT", spec, subdim=False, uops_sha={})
    _register_op(op)
    return op


def _register_recip_mul():
    import concourse.dve_ops as dve_ops
    from concourse.dve_ops import DveOp
    from concourse.dve_spec import Spec, Src0, Src1, C0, C1, Bin, AluOp

    for op in dve_ops.OPS:
        if op.name == "RECIP1_MUL_ANT":
            return op

    def _ref(in0, in1, s0, s1, imm2):
        x = np.ascontiguousarray(in0, dtype=np.float32)
        nx = (~x.view(np.int32)).view(np.float32)
        y0 = nx * np.float32(s0)
        y1 = y0 * (np.float32(s1) - x * y0)
        return y1 * in1

    nx = Bin(AluOp.BITWISE_NOT, Src0, Src0)
    y0 = nx * C0
    y1 = y0 * (C1 - Src0 * y0)
    spec = Spec(body=y1 * Src1, reference=_ref)
    op = DveOp("RECIP1_MUL_ANT", spec, subdim=False, uops_sha={})
    _register_op(op)
    return op


def _register_tanh_mul():
    import concourse.dve_ops as dve_ops
    from concourse.dve_ops import DveOp
    from concourse.dve_spec import Spec, Src0, Src1, C0, C1, C2

    for op in dve_ops.OPS:
        if op.name == "TANH_MUL_ANT":
            return op
    u = Src0
    x2 = u * u
    x4 = x2 * x2
    t = u * (C0 + C1 * x2 + C2 * x4)
    spec = Spec(
        body=t * Src1,
        reference=lambda in0, in1, s0, s1, imm2:
            in0 * (s0 + s1 * in0 * in0 + imm2 * in0**4) * in1,
    )
    op = DveOp("TANH_MUL_ANT", spec, subdim=False, uops_sha={})
    _register_op(op)
    return op


# ---------------------------------------------------------------------------
# Device kernel (Bass / Tile).

_BUILD_CACHE = {}

POOLCFG = {"pgf": 3, "pgb": 3, "pc": 1, "pd": 1, "spool": 4, "xpool": 5,
           "xraw": 3, "opool": 2, "group": 4}


def _build(n_samples):
    """Build + compile the Bacc program for one core processing n_samples."""
    key = (n_samples, tuple(sorted(POOLCFG.items())))
    if key in _BUILD_CACHE:
        return _BUILD_CACHE[key]

    import concourse.bass as bass
    import concourse.mybir as mybir
    from concourse import bacc
    from concourse.tile import TileContext
    from concourse.dve_ops import RECIPROCAL_APPROX_FAST, RECIP_APPROX_FAST_CONSTS

    EXP4 = _register_exp4()
    TMUL = _register_tanh_mul()
    RMUL = _register_recip_mul()
    RC = RECIP_APPROX_FAST_CONSTS

    bf16 = mybir.dt.bfloat16
    f32 = mybir.dt.float32
    i8 = mybir.dt.int8
    u8 = mybir.dt.uint8
    AF = mybir.ActivationFunctionType
    Alu = mybir.AluOpType
    Sig, Tanh = AF.Sigmoid, AF.Tanh

    conv_order = list(CONV_COLS.keys())
    conv_off = {}
    c = 0
    for k in conv_order:
        conv_off[k] = c
        c += CONV_COLS[k]

    # merged weight tensor column offsets
    OFF_WIH = 0
    OFF_WHH = N_WIH
    OFF_CONV = N_WIH + N_WHH
    OFF_ONES = OFF_CONV + N_CONV
    OFF_BIAS = OFF_ONES + N_ONES
    ones_off = {8: 0, 16: 128, 32: 256, 64: 384}

    wih_names = []
    for d in ("f", "b"):
        for lvl in (1, 2, 3, 4):
            first = (d == "f" and lvl == 1) or (d == "b" and lvl == 4)
            last = d == "b" and lvl == 1
            gates = ("i", "g", "o") if first else (("i", "f", "g") if last else GATES)
            for gt in gates:
                wih_names.append(f"{d}{lvl}_{gt}")
    wih_off = {k: 64 * i for i, k in enumerate(wih_names)}
    bias_off = {k: i for i, k in enumerate(wih_names)}

    whh_names = [f"f{l}_{g}" for l in (2, 3, 4) for g in GATES]
    whh_names += [f"b{l}_{g}" for l in (3, 2) for g in GATES]
    whh_names += [f"b1_{g}" for g in ("i", "f", "g")]
    whh_off = {k: 64 * i for i, k in enumerate(whh_names)}
    whh_K = {}
    for k in whh_names:
        d, lvl = k[0], int(k[1])
        whh_K[k] = 32 if d == "f" else 64

    npairs = n_samples // (2 * NT)
    nlow = npairs // 2          # pairs with columns in the low-nibble half
    nq = n_samples // 4         # columns per outT quarter tensor

    nc = bacc.Bacc("TRN2", target_bir_lowering=False, debug=False)
    xhi = nc.dram_tensor("xhi", (128, n_samples), i8, kind="ExternalInput")
    xnib = nc.dram_tensor("xnib", (128, n_samples // 2), u8,
                          kind="ExternalInput")
    wall_d = nc.dram_tensor("wall", (128, N_WALL), bf16, kind="ExternalInput")
    outq = [nc.dram_tensor(f"outq{i}", (128, nq), i8, kind="ExternalOutput")
            for i in range(4)]
    oscale = nc.dram_tensor("oscale", (128, 2 * npairs), f32,
                            kind="ExternalOutput")

    xhi_ap = xhi.ap()
    xnib_ap = xnib.ap()
    outq_ap = [t.ap() for t in outq]
    oscale_ap = oscale.ap()

    with TileContext(nc) as tc:
        with (
            tc.tile_pool(name="wpool", bufs=1) as wpool,
            tc.tile_pool(name="xpool", bufs=POOLCFG["xpool"]) as xpool,
            tc.tile_pool(name="xraw", bufs=POOLCFG["xraw"]) as xraw,
            tc.tile_pool(name="spool", bufs=POOLCFG["spool"]) as spool,
            tc.tile_pool(name="opool", bufs=POOLCFG["opool"]) as opool,
            tc.tile_pool(name="qpool", bufs=4) as qpool,
            tc.tile_pool(name="pgf", bufs=POOLCFG["pgf"], space="PSUM") as pgf,
            tc.tile_pool(name="pgb", bufs=POOLCFG["pgb"], space="PSUM") as pgb,
            tc.tile_pool(name="pc", bufs=POOLCFG["pc"], space="PSUM") as pc,
            tc.tile_pool(name="pd", bufs=POOLCFG["pd"], space="PSUM") as pd,
        ):
            wall_sb = wpool.tile([128, N_WALL], bf16)
            nc.sync.dma_start(out=wall_sb[:], in_=wall_d.ap()[:, :])

            def wih_sl(name):
                o = OFF_WIH + wih_off[name]
                return wall_sb[:, o:o + 64]

            def whh_sl(base, name):
                o = OFF_WHH + whh_off[name]
                K = whh_K[name]
                return wall_sb[base:base + K, o:o + 64]

            def conv_sl(rows, name, width):
                o = OFF_CONV + conv_off[name]
                return wall_sb[rows[0]:rows[1], o:o + width]

            def ones_sl(width):
                o = OFF_ONES + ones_off[width]
                return wall_sb[:, o:o + 128]

            def bias_sl(name):
                o = OFF_BIAS + bias_off[name]
                return wall_sb[:, o:o + 1]

            def exp4(out_ap, in_ap):
                nc.vector._custom_dve(EXP4, out=out_ap, in0=in_ap,
                                      s0=EXP_C1, s1=EXP_C2, imm2=EXP_C3)

            def tanh_mul(c_ap, s_ap, tag, out=None):
                if out is None:
                    out = spool.tile([128, NT], bf16, tag=tag, name=tag)[:]
                nc.vector._custom_dve(TMUL, out=out, in0=c_ap, in1=s_ap,
                                      s0=TANH_A, s1=TANH_B, imm2=TANH_C)
                return out

            def gates_mm(d, lvl, gates, XA, XB, hbA=None, hbB=None):
                """gate -> psum tile [128, NT]; XA/XB are (hi, lo) pairs."""
                XAhi, XAlo = XA
                XBhi, XBlo = XB
                ps = {}
                pool = pgf if d == "f" else pgb
                for gt in gates:
                    name = f"{d}{lvl}_{gt}"
                    p = pool.tile([128, NT], f32, tag="gates", name="gates")
                    w = wih_sl(name)
                    has_hh = hbA is not None and name in whh_off
                    if has_hh:
                        wA = whh_sl(hbA.base_partition(), name)
                        wB = whh_sl(hbB.base_partition(), name)
                    # NB: each half's accumulation group must close before the
                    # other half's opens — PSUM zero-regions are bank-granular.
                    nc.tensor.matmul(p[0:64, :], w, XAhi[:],
                                     start=True, stop=False)
                    nc.tensor.matmul(p[0:64, :], w, XAlo[:],
                                     start=False, stop=not has_hh)
                    if has_hh:
                        nc.tensor.matmul(p[0:64, :], wA, hbA,
                                         start=False, stop=True)
                    nc.tensor.matmul(p[64:128, :], w, XBhi[:],
                                     start=True, stop=False)
                    nc.tensor.matmul(p[64:128, :], w, XBlo[:],
                                     start=False, stop=not has_hh)
                    if has_hh:
                        nc.tensor.matmul(p[64:128, :], wB, hbB,
                                         start=False, stop=True)
                    ps[gt] = p
                return ps

            def act(func, ps_tile, d, lvl, gt):
                name = f"{d}{lvl}_{gt}"
                o = spool.tile([128, NT], bf16, tag=f"a_{gt}")
                nc.scalar.activation(out=o[:], in_=ps_tile[:], func=func,
                                     bias=bias_sl(name), scale=1.0)
                return o

            def tanh_sbuf(t_in, tag):
                o = spool.tile([128, NT], bf16, tag=tag)
                nc.scalar.activation(out=o[:], in_=t_in[:], func=Tanh)
                return o

            def tt(op, a, b, tag=None, out=None):
                if out is None:
                    out = spool.tile([128, NT], bf16, tag=tag, name=tag)[:]
                if op == "mul":
                    nc.vector.tensor_mul(out, a, b)
                else:
                    nc.vector.tensor_add(out, a, b)
                return out

            def quant_store(v, p, which):
                """Quantize tile v [128, NT x2-half layout] to int8 with
                per-row scale; store to outq + oscale column 2p+which."""
                m = qpool.tile([128, 1], f32, tag=f"m{which}")
                nc.vector.tensor_reduce(out=m[:], in_=v, axis=mybir.AxisListType.X,
                                        op=Alu.max, apply_absolute_value=True)
                m2 = qpool.tile([128, 1], f32, tag=f"m2{which}")
                nc.vector.tensor_scalar_max(m2[:], m[:], 1e-8)
                inv = qpool.tile([128, 1], f32, tag=f"inv{which}")
                nc.vector.reciprocal(out=inv[:], in_=m2[:])
                q = qpool.tile([128, NT], i8, tag=f"q{which}")
                nc.vector.tensor_scalar(out=q[:], in0=v, scalar1=inv[:],
                                        scalar2=127.0, op0=Alu.mult,
                                        op1=Alu.mult)
                ti = (2 * p * NT) // nq
                lo = 2 * p * NT - ti * nq
                rows = (0, 64) if which == 0 else (64, 128)
                nc.sync.dma_start(out=outq_ap[ti][rows[0]:rows[1],
                                                  lo:lo + NT],
                                  in_=q[0:64, :])
                nc.sync.dma_start(out=outq_ap[ti][rows[0]:rows[1],
                                                  lo + NT:lo + 2 * NT],
                                  in_=q[64:128, :])
                nc.gpsimd.dma_start(out=oscale_ap[:, 2 * p + which:2 * p + which + 1],
                                    in_=m2[:])

            def trans_fwd(stage, h_pair, c_pair, width):
                e_ps = pc.tile([128, NT], f32, tag="eps")
                nc.tensor.matmul(e_ps[0:32, :], conv_sl((0, 64), stage + "h", 32),
                                 h_pair[0:64, :], start=True, stop=True,
                                 tile_position=(0, 0))
                nc.tensor.matmul(e_ps[32:64, :], conv_sl((0, 64), stage + "c", 32),
                                 c_pair[0:64, :], start=True, stop=True,
                                 tile_position=(0, 32))
                nc.tensor.matmul(e_ps[64:96, :], conv_sl((64, 128), stage + "h", 32),
                                 h_pair[64:128, :], start=True, stop=True,
                                 tile_position=(64, 64))
                nc.tensor.matmul(e_ps[96:128, :], conv_sl((64, 128), stage + "c", 32),
                                 c_pair[64:128, :], start=True, stop=True,
                                 tile_position=(64, 96))
                e = spool.tile([128, NT], bf16, tag="e")
                exp4(e[:], e_ps[:])
                d_ps = pd.tile([128, NT], f32, tag="dps")
                nc.tensor.matmul(d_ps[:], ones_sl(width), e[:],
                                 start=True, stop=True)
                nb = spool.tile([128, NT], bf16, tag="nb_" + stage)
                nc.vector._custom_dve(RMUL, out=nb[:], in0=d_ps[:], in1=e[:],
                                      s0=RC["s0"], s1=RC["s1"])
                return nb

            def trans_bwd(stage, h_pair, c_pair, width):
                nbs = []
                for which in ("A", "B"):
                    e_ps = pc.tile([128, NT], f32, tag="eps")
                    if which == "A":
                        rh, rc = h_pair[0:64, :], c_pair[0:64, :]
                        wrows = (0, 64)
                        c_rows, h_rows = (0, 64), (64, 128)
                    else:
                        rh, rc = h_pair[64:128, :], c_pair[64:128, :]
                        wrows = (64, 128)
                        c_rows, h_rows = (64, 128), (0, 64)
                    nc.tensor.matmul(e_ps[c_rows[0]:c_rows[1], :],
                                     conv_sl(wrows, stage + "c", 64), rc,
                                     start=True, stop=True)
                    nc.tensor.matmul(e_ps[h_rows[0]:h_rows[1], :],
                                     conv_sl(wrows, stage + "h", 64), rh,
                                     start=True, stop=True)
                    e = spool.tile([128, NT], bf16, tag="e")
                    exp4(e[:], e_ps[:])
                    d_ps = pd.tile([128, NT], f32, tag="dps")
                    nc.tensor.matmul(d_ps[:], ones_sl(width), e[:],
                                     start=True, stop=True)
                    nb = spool.tile([128, NT], bf16, tag=f"nb{which}_" + stage)
                    nc.vector._custom_dve(RMUL, out=nb[:], in0=d_ps[:],
                                          in1=e[:], s0=RC["s0"], s1=RC["s1"])
                    nbs.append(nb)
                return nbs

            def fwd_chain(XA, XB, p):
                G = gates_mm("f", 1, ("i", "g", "o"), XA, XB)
                sI = act(Sig, G["i"], "f", 1, "i")
                sO = act(Sig, G["o"], "f", 1, "o")
                tG = act(Tanh, G["g"], "f", 1, "g")
                c1 = tt("mul", sI[:], tG[:], tag="cst")
                h1 = tanh_mul(c1, sO[:], "h")
                yield
                nb12 = trans_fwd("f12", h1, c1, 32)

                cbd = spool.tile([128, NT], bf16, tag="cbd")
                nc.sync.dma_start(out=cbd[0:32, :], in_=nb12[32:64, :])
                nc.gpsimd.dma_start(out=cbd[32:64, :], in_=nb12[32:64, :])
                nc.sync.dma_start(out=cbd[64:96, :], in_=nb12[96:128, :])
                nc.gpsimd.dma_start(out=cbd[96:128, :], in_=nb12[96:128, :])
                yield

                G = gates_mm("f", 2, GATES, XA, XB,
                             nb12[0:32, :], nb12[64:96, :])
                sI = act(Sig, G["i"], "f", 2, "i")
                sF = act(Sig, G["f"], "f", 2, "f")
                sO = act(Sig, G["o"], "f", 2, "o")
                tG = act(Tanh, G["g"], "f", 2, "g")
                t1 = tt("mul", sI[:], tG[:], tag="t1")
                t2 = tt("mul", sF[:], cbd[:], tag="t2")
                c2 = tt("add", t1, t2, tag="cst")
                h2 = tanh_mul(c2, sO[:], "h")
                yield
                nb23 = trans_fwd("f23", h2, c2, 16)

                cbd = spool.tile([128, NT], bf16, tag="cbd")
                for base, src in ((0, 32), (64, 96)):
                    for s in range(4):
                        blk = src + 16 * (s // 2)
                        eng = nc.sync if s % 2 else nc.gpsimd
                        eng.dma_start(
                            out=cbd[base + 16 * s:base + 16 * s + 16, :],
                            in_=nb23[blk:blk + 16, :])
                yield

                G = gates_mm("f", 3, GATES, XA, XB,
                             nb23[0:32, :], nb23[64:96, :])
                sI = act(Sig, G["i"], "f", 3, "i")
                sF = act(Sig, G["f"], "f", 3, "f")
                sO = act(Sig, G["o"], "f", 3, "o")
                tG = act(Tanh, G["g"], "f", 3, "g")
                t1 = tt("mul", sI[:], tG[:], tag="t1")
                t2 = tt("mul", sF[:], cbd[:], tag="t2")
                c3 = tt("add", t1, t2, tag="cst")
                h3 = tanh_mul(c3, sO[:], "h")
                yield
                nb34 = trans_fwd("f34", h3, c3, 8)

                cbd = spool.tile([128, NT], bf16, tag="cbd")
                for base, src in ((0, 32), (64, 96)):
                    for s in range(8):
                        blk = src + 8 * (s // 2)
                        eng = nc.sync if s % 2 else nc.gpsimd
                        eng.dma_start(
                            out=cbd[base + 8 * s:base + 8 * s + 8, :],
                            in_=nb34[blk:blk + 8, :])
                yield

                G = gates_mm("f", 4, GATES, XA, XB,
                             nb34[0:32, :], nb34[64:96, :])
                sI = act(Sig, G["i"], "f", 4, "i")
                sF = act(Sig, G["f"], "f", 4, "f")
                sO = act(Sig, G["o"], "f", 4, "o")
                tG = act(Tanh, G["g"], "f", 4, "g")
                t1 = tt("mul", sI[:], tG[:], tag="t1")
                t2 = tt("mul", sF[:], cbd[:], tag="t2")
                c4 = tt("add", t1, t2, tag="cst")
                tC = tanh_sbuf(c4, "tC")
                outh = opool.tile([128, NT], bf16, tag="outh")
                tt("mul", sO[:], tC[:], out=outh[:])
                quant_store(outh[:], p, 0)

            def bwd_chain(XA, XB, p):
                G = gates_mm("b", 4, ("i", "g", "o"), XA, XB)
                sI = act(Sig, G["i"], "b", 4, "i")
                sO = act(Sig, G["o"], "b", 4, "o")
                tG = act(Tanh, G["g"], "b", 4, "g")
                c4b = tt("mul", sI[:], tG[:], tag="cstb")
                h4b = tanh_mul(c4b, sO[:], "hb")
                yield
                nbA, nbB = trans_bwd("b43", h4b, c4b, 16)
                yield

                G = gates_mm("b", 3, GATES, XA, XB,
                             nbA[64:128, :], nbB[0:64, :])
                sI = act(Sig, G["i"], "b", 3, "i")
                sF = act(Sig, G["f"], "b", 3, "f")
                sO = act(Sig, G["o"], "b", 3, "o")
                tG = act(Tanh, G["g"], "b", 3, "g")
                t1 = tt("mul", sI[:], tG[:], tag="t1b")
                t2b = spool.tile([128, NT], bf16, tag="t2b", name="t2b")
                nc.vector.tensor_mul(t2b[0:64, :], sF[0:64, :],
                                     nbA[0:64, :])
                nc.vector.tensor_mul(t2b[64:128, :], sF[64:128, :],
                                     nbB[64:128, :])
                c3b = tt("add", t1, t2b[:], tag="cstb")
                h3b = tanh_mul(c3b, sO[:], "hb")
                yield
                nbA, nbB = trans_bwd("b32", h3b, c3b, 32)
                yield

                G = gates_mm("b", 2, GATES, XA, XB,
                             nbA[64:128, :], nbB[0:64, :])
                sI = act(Sig, G["i"], "b", 2, "i")
                sF = act(Sig, G["f"], "b", 2, "f")
                sO = act(Sig, G["o"], "b", 2, "o")
                tG = act(Tanh, G["g"], "b", 2, "g")
                t1 = tt("mul", sI[:], tG[:], tag="t1b")
                t2b = spool.tile([128, NT], bf16, tag="t2b", name="t2b")
                nc.vector.tensor_mul(t2b[0:64, :], sF[0:64, :],
                                     nbA[0:64, :])
                nc.vector.tensor_mul(t2b[64:128, :], sF[64:128, :],
                                     nbB[64:128, :])
                c2b = tt("add", t1, t2b[:], tag="cstb")
                h2b = tanh_mul(c2b, sO[:], "hb")
                yield
                nbA, nbB = trans_bwd("b21", h2b, c2b, 64)
                yield

                G = gates_mm("b", 1, ("i", "f", "g"), XA, XB,
                             nbA[64:128, :], nbB[0:64, :])
                sI = act(Sig, G["i"], "b", 1, "i")
                sF = act(Sig, G["f"], "b", 1, "f")
                tG = act(Tanh, G["g"], "b", 1, "g")
                t1 = tt("mul", sI[:], tG[:], tag="t1b")
                t2b = spool.tile([128, NT], bf16, tag="t2b", name="t2b")
                nc.vector.tensor_mul(t2b[0:64, :], sF[0:64, :],
                                     nbA[0:64, :])
                nc.vector.tensor_mul(t2b[64:128, :], sF[64:128, :],
                                     nbB[64:128, :])
                outc = opool.tile([128, NT], bf16, tag="outc")
                tt("add", t1, t2b[:], out=outc[:])
                quant_store(outc[:], p, 1)

            import itertools

            def decode_half(tag, hi_cols, nib_cols, high_nibble):
                """Load + decode one NT-column half: returns (Xhi, Xlo) bf16."""
                hi8 = xraw.tile([128, NT], i8, tag=f"hi8{tag}")
                nc.sync.dma_start(out=hi8[:], in_=xhi_ap[:, hi_cols])
                nib = xraw.tile([128, NT], u8, tag=f"nib{tag}")
                nc.sync.dma_start(out=nib[:], in_=xnib_ap[:, nib_cols])
                Xhi = xpool.tile([128, NT], bf16, tag=f"Xhi{tag}")
                nc.scalar.activation(out=Xhi[:], in_=hi8[:], func=AF.Copy)
                lo8 = xraw.tile([128, NT], u8, tag=f"lo8{tag}")
                if high_nibble:
                    nc.vector.tensor_scalar(out=lo8[:], in0=nib[:], scalar1=4,
                                            op0=Alu.logical_shift_right,
                                            scalar2=None)
                else:
                    nc.vector.tensor_scalar(out=lo8[:], in0=nib[:], scalar1=15,
                                            op0=Alu.bitwise_and, scalar2=None)
                Xlo = xpool.tile([128, NT], bf16, tag=f"Xlo{tag}")
                nc.scalar.activation(out=Xlo[:], in_=lo8[:], func=AF.Copy,
                                     scale=0.0625)
                return Xhi, Xlo

            def pair_chains(p):
                cA = slice(2 * p * NT, 2 * p * NT + NT)
                cB = slice(2 * p * NT + NT, 2 * p * NT + 2 * NT)
                high = p >= nlow
                pn = p - nlow if high else p
                nA = slice(2 * pn * NT, 2 * pn * NT + NT)
                nB = slice(2 * pn * NT + NT, 2 * pn * NT + 2 * NT)
                XA = decode_half("A", cA, nA, high)
                XB = decode_half("B", cB, nB, high)
                return (fwd_chain(XA, XB, p), bwd_chain(XA, XB, p))

            GROUP = POOLCFG.get("group", 1)
            for p0 in range(0, npairs, GROUP):
                chains = []
                for p in range(p0, min(p0 + GROUP, npairs)):
                    chains.extend(pair_chains(p))
                for _ in itertools.zip_longest(*chains):
                    pass

    nc.compile()
    _BUILD_CACHE[key] = nc
    return nc


# ---------------------------------------------------------------------------
# Fast single-core runner: cached jit around the bass_exec primitive.

_RUNNER_CACHE = {}


def _make_runner(nc):
    if id(nc) in _RUNNER_CACHE:
        return _RUNNER_CACHE[id(nc)]

    import jax
    import jax.numpy as jnp
    import concourse.mybir as mybir
    from concourse import bass2jax

    bass2jax.install_neuronx_cc_hook()

    in_names, out_names, out_avals = [], [], []
    pname = nc.partition_id_tensor.name if nc.partition_id_tensor else None
    for alloc in nc.m.functions[0].allocations:
        if not isinstance(alloc, mybir.MemoryLocationSet):
            continue
        name = alloc.memorylocations[0].name
        if alloc.kind == "ExternalInput":
            if name != pname:
                in_names.append(name)
        elif alloc.kind == "ExternalOutput":
            out_names.append(name)
            out_avals.append(jax.core.ShapedArray(
                tuple(alloc.tensor_shape), mybir.dt.np(alloc.dtype)))

    all_names = list(in_names) + list(out_names)
    if pname is not None:
        all_names.append(pname)

    def _body(*args):
        operands = list(args)
        if pname is not None:
            operands.append(bass2jax.partition_id_tensor())
        outs = bass2jax._bass_exec_p.bind(
            *operands,
            out_avals=tuple(out_avals),
            in_names=tuple(all_names),
            out_names=tuple(out_names),
            lowering_input_output_aliases=(),
            sim_require_finite=True,
            sim_require_nnan=True,
            nc=nc,
        )
        return tuple(outs)

    jitted = jax.jit(_body, keep_unused=True)
    # device-resident zero buffers for the output operands: created once on
    # device (never shipped over the tunnel), reused read-only every call.
    dev = jax.devices()[0]
    zeros = jax.jit(
        lambda: tuple(jnp.zeros(a.shape, a.dtype) for a in out_avals),
        device=dev)()
    runner = (jitted, in_names, out_names, list(zeros))
    _RUNNER_CACHE[id(nc)] = runner
    return runner


def _run_device(nc, in_map):
    """Threaded upload -> single jit call -> threaded fetch.

    Returns dict name -> np.ndarray. This is the full device round trip
    (host arrays in, host arrays out) for one run.
    """
    import jax

    jitted, in_names, out_names, zeros = _make_runner(nc)
    dev = jax.devices()[0]

    # threaded upload of all inputs
    staged = {}

    def put(name):
        x = jax.device_put(in_map[name], dev)
        x.block_until_ready()
        staged[name] = x

    threads = [threading.Thread(target=put, args=(n,)) for n in in_names]
    for t in threads:
        t.start()
    for t in threads:
        t.join()

    outs = jitted(*[staged[n] for n in in_names], *zeros)
    res = {}

    def get(i, name):
        res[name] = np.asarray(outs[i])

    threads = [threading.Thread(target=get, args=(i, n))
               for i, n in enumerate(out_names)]
    for t in threads:
        t.start()
    for t in threads:
        t.join()
    return res


# ---------------------------------------------------------------------------
# Host quantize / dequantize.

def quantize_input(spec):
    """spec [B, D] f32 -> (xhi int8 [128, B], xnib u8 [128, B//2], amax)."""
    spec = np.asarray(spec, np.float32)
    amax = float(np.abs(spec).max())
    n = spec.shape[0]
    xT = spec.T                                  # [128, B] view
    q = np.rint(xT * (QBITS_MAX / amax)).astype(np.int32)
    np.clip(q, -QBITS_MAX, QBITS_MAX, out=q)
    hi = (q >> 4).astype(np.int8)
    lo = (q & 15).astype(np.uint8)
    n2 = n // 2
    xnib = (lo[:, :n2] | (lo[:, n2:] << 4)).astype(np.uint8)
    return np.ascontiguousarray(hi), np.ascontiguousarray(xnib), amax


def dequantize_output(res, n):
    """res: dict with outq0..3 int8 [128, n//4] + oscale f32 [128, 2*npairs].
    Returns [n, 128] float32."""
    npairs = n // (2 * NT)
    q = np.concatenate([np.asarray(res[f"outq{i}"]) for i in range(4)], axis=1)
    sc = np.asarray(res["oscale"], np.float32)      # [128, 2*npairs]
    mh = sc[:, 0::2]                                # [128, npairs]
    mc = sc[:, 1::2]
    # element [r, pair p, half h, col] has scale:
    #   r <  64: mh[64*h + r, p]        (outh rows)
    #   r >= 64: mc[64*h + (r-64), p]   (outc rows)
    S = np.empty((128, npairs, 2), np.float32)
    for h in (0, 1):
        S[:64, :, h] = mh[64 * h:64 * h + 64, :]
        S[64:, :, h] = mc[64 * h:64 * h + 64, :]
    out = q.astype(np.float32).reshape(128, npairs, 2, NT)
    out *= (S / 127.0)[:, :, :, None]
    return np.ascontiguousarray(out.reshape(128, n).T)


def _bf16(a):
    return np.ascontiguousarray(a).astype(ml_dtypes.bfloat16)


def prepare_inputs(inputs):
    """Host prep: pack weights + quantize spec. Returns (nc, in_map, n)."""
    spec = np.asarray(inputs["spec"], np.float32)
    n = spec.shape[0]
    xhi, xnib, amax = quantize_input(spec)
    # x_hat = (amax/127) * (hi + lo/16); fold amax/127 into wih
    pack = Pack(inputs, xscale=amax / 127.0)
    nc = _build(n)
    in_map = {"xhi": xhi, "xnib": xnib, "wall": np.ascontiguousarray(pack.wall())}
    return nc, in_map, n


def kernel(**inputs):
    nc, in_map, n = prepare_inputs(inputs)
    res = _run_device(nc, in_map)
    return dequantize_output(res, n)


if __name__ == "__main__":
    pass


# revision 12
# speedup vs baseline: 2.6876x; 1.0430x over previous
"""Trainium2 Bass kernel for nn_DSGSF (batched bidirectional multi-scale LSTM).

Transfer-optimized design. The axon tunnel moves ~65MB/s up / ~47MB/s down and
serializes all traffic, while the device itself needs only ~5ms — so the
kernel minimizes tunnel bytes and RPC count:

  - input spec is shipped as a 12-bit quantization: an int8 "hi" plane plus a
    packed-nibble "lo" plane (1.5 B/sample-feature instead of 2 B for bf16;
    also ~4x less quantization noise than bf16's 8-bit mantissa);
  - the quantization scale is folded into the host-packed W_ih matrices, and
    gates are computed as (W')*hi + (W')*(lo/16) with two bf16 matmuls
    sharing one stationary weight;
  - output is shipped as int8 with per-row per-tile scales computed on
    device (absmax reduce -> reciprocal -> scale+convert), plus a tiny
    [128, 2*npairs] f32 scale tensor;
  - everything runs on ONE core (device compute is ~1000x faster than the
    tunnel; splitting across cores only multiplies per-transfer overhead);
  - a cached jax.jit around the bass_exec primitive avoids per-call
    retrace/relower, and output operand buffers are created on-device
    (jnp.zeros) instead of being shipped from host.

On-device math is unchanged from the baseline: transposed layout
(features/gates on partitions, samples on the free dim), block-diagonal
lhsT matmuls per gate covering all steps of a level, conv1d band matrices
with the exp 1/4 prescale folded in, block-ones matmuls for softmax
denominators, sigmoid/tanh on ACT, exp as a fused cubic-poly DVE op,
reciprocal via RECIPROCAL_APPROX_FAST.
"""

import threading

import numpy as np
import ml_dtypes

B, D, H = 131072, 128, 64
NT = 512                 # samples per tile (one PSUM bank)
QBITS_MAX = 2032         # 12-bit quant: q in [-2032, 2032], hi=q>>4, lo=q&15

GATES = ("i", "f", "g", "o")

# ---------------------------------------------------------------------------
# exp4 polynomial: exp(y) = p(y/4)^4 with p(u) = 1 + c1 u + c2 u^2 + c3 u^3
# fitted for relative error of p(u)^4 vs e^(4u) on |u| <= EXP_FIT_RANGE.
EXP_FIT_RANGE = 0.85


def _fit_exp_poly():
    u = np.linspace(-EXP_FIT_RANGE, EXP_FIT_RANGE, 4001)
    A = np.stack([u, u * u, u**3], axis=1) / np.exp(u)[:, None]
    b = (np.exp(u) - 1.0) / np.exp(u)
    c, *_ = np.linalg.lstsq(A, b, rcond=None)
    return c


EXP_C1, EXP_C2, EXP_C3 = (float(v) for v in _fit_exp_poly())


def _fit_tanh5():
    u = np.linspace(-2.05, 2.05, 4001)
    A = np.stack([u, u**3, u**5], axis=1)
    c, *_ = np.linalg.lstsq(A, np.tanh(u), rcond=None)
    return c


TANH_A, TANH_B, TANH_C = (float(v) for v in _fit_tanh5())


def tanh5_np(u):
    return u * (TANH_A + TANH_B * u * u + TANH_C * u**4)


def exp4_np(y):
    u = y.astype(np.float64)
    p = 1.0 + u * (EXP_C1 + u * (EXP_C2 + EXP_C3 * u))
    return (p * p) ** 2


def recip_np(x):
    x = x.astype(np.float32)
    nx = (~x.view(np.int32)).view(np.float32)
    y0 = nx * np.float32(-0.23549792)
    y1 = y0 * (np.float32(2.0017324) - x * y0)
    return y1 * (np.float32(2.0) - x * y1)


def _sigmoid(x):
    return 1.0 / (1.0 + np.exp(-x))


# ---------------------------------------------------------------------------
# Host-side weight packing.
LEVEL_STEPS = {1: 1, 2: 2, 3: 4, 4: 8}


def _ih_lhsT(w_ih, level, gate):
    """Block-diagonal lhsT [128, 64] computing `gate` preacts for all steps."""
    Sl = LEVEL_STEPS[level]
    dh, di = 64 // Sl, 128 // Sl
    gi = GATES.index(gate)
    wg = w_ih[gi * dh:(gi + 1) * dh, :]
    out = np.zeros((128, 64), np.float32)
    for s in range(Sl):
        out[s * di:(s + 1) * di, s * dh:(s + 1) * dh] = wg.T
    return out


def _hh_lhsT(w_hh, level, gate, direction):
    """lhsT [K, 64] mapping hb rows onto gate preacts for all steps."""
    Sl = LEVEL_STEPS[level]
    dh = 64 // Sl
    dprev = w_hh.shape[1]
    gi = GATES.index(gate)
    wg = w_hh[gi * dh:(gi + 1) * dh, :]
    if direction == "f":
        nblk = {2: 1, 3: 2, 4: 4}[level]
        K = nblk * dprev
        out = np.zeros((K, 64), np.float32)
        for s in range(Sl):
            blk = s * nblk // Sl
            out[blk * dprev:(blk + 1) * dprev, s * dh:(s + 1) * dh] = wg.T
    else:
        K = Sl * dprev
        out = np.zeros((K, 64), np.float32)
        for s in range(Sl):
            out[s * dprev:(s + 1) * dprev, s * dh:(s + 1) * dh] = wg.T
    return out


def _conv_lhsT(w3, win, nblk, stride, scale):
    """Band matrix [nblk*win, nblk*wout] for blockwise conv1d(k=3,pad=1)."""
    w = np.asarray(w3, np.float64).reshape(3) * scale
    wout = win // stride
    blk = np.zeros((win, wout), np.float64)
    for j in range(wout):
        for t in range(3):
            k = stride * j - 1 + t
            if 0 <= k < win:
                blk[k, j] += w[t]
    out = np.zeros((nblk * win, nblk * wout), np.float32)
    for q in range(nblk):
        out[q * win:(q + 1) * win, q * wout:(q + 1) * wout] = blk
    return out


def _ones_block(width):
    out = np.zeros((128, 128), np.float32)
    for q in range(128 // width):
        out[q * width:(q + 1) * width, q * width:(q + 1) * width] = 1.0
    return out


def _bias_pair(b, level, gate):
    """[128] bias rows for a pair-packed gate psum tile (same 64 twice)."""
    Sl = LEVEL_STEPS[level]
    dh = 64 // Sl
    gi = GATES.index(gate)
    bg = np.asarray(b, np.float32)[gi * dh:(gi + 1) * dh]
    one = np.tile(bg, Sl)
    return np.concatenate([one, one])


class Pack:
    """All host-packed constant matrices + column offset maps.

    xscale is folded into the wih blocks: device rhs is (hi + lo/16) and
    x_hat = xscale * (hi + lo/16), so wih' = wih * xscale.
    """

    def __init__(self, inp, xscale=1.0):
        g = lambda n: np.asarray(inp[n], np.float32)

        self.wih = {}
        order = []
        for d in ("f", "b"):
            for lvl in (1, 2, 3, 4):
                w = g(f"{d}w_ih{lvl}")
                first = (d == "f" and lvl == 1) or (d == "b" and lvl == 4)
                last = d == "b" and lvl == 1
                gates = ("i", "g", "o") if first else (("i", "f", "g") if last else GATES)
                for gt in gates:
                    self.wih[f"{d}{lvl}_{gt}"] = _ih_lhsT(w, lvl, gt) * xscale
                    order.append(f"{d}{lvl}_{gt}")
        self.wih_order = order
        self.wih_mat = np.concatenate([self.wih[k] for k in order], axis=1)

        # hh lhsT blocks, replicated at partition offset 64 for B-tile matmuls
        self.whh = {}
        horder = []
        for lvl in (2, 3, 4):
            w = g(f"fw_hh{lvl}")
            for gt in GATES:
                self.whh[f"f{lvl}_{gt}"] = _hh_lhsT(w, lvl, gt, "f")
                horder.append(f"f{lvl}_{gt}")
        for lvl in (3, 2, 1):
            w = g(f"bw_hh{lvl}")
            gates = ("i", "f", "g") if lvl == 1 else GATES
            for gt in gates:
                self.whh[f"b{lvl}_{gt}"] = _hh_lhsT(w, lvl, gt, "b")
                horder.append(f"b{lvl}_{gt}")
        self.whh_order = horder
        self.whh_K = {k: self.whh[k].shape[0] for k in horder}
        mats = []
        for k in horder:
            m = np.zeros((128, 64), np.float32)
            m[: self.whh_K[k], :] = self.whh[k]
            m[64:64 + self.whh_K[k], :] = self.whh[k]
            mats.append(m)
        self.whh_mat = np.concatenate(mats, axis=1)

        sc = 0.25
        self.wconv = {
            "f12h": _conv_lhsT(g("ft12h"), 64, 1, 2, sc),
            "f12c": _conv_lhsT(g("ft12c"), 64, 1, 2, sc),
            "f23h": _conv_lhsT(g("ft23h"), 32, 2, 2, sc),
            "f23c": _conv_lhsT(g("ft23c"), 32, 2, 2, sc),
            "f34h": _conv_lhsT(g("ft34h"), 16, 4, 2, sc),
            # NB: reference reuses ft34h for the c path (original model bug)
            "f34c": _conv_lhsT(g("ft34h"), 16, 4, 2, sc),
            "b43h": _conv_lhsT(g("bt43h"), 16, 4, 1, sc),
            "b43c": _conv_lhsT(g("bt43c"), 16, 4, 1, sc),
            "b32h": _conv_lhsT(g("bt32h"), 32, 2, 1, sc),
            "b32c": _conv_lhsT(g("bt32c"), 32, 2, 1, sc),
            "b21h": _conv_lhsT(g("bt21h"), 64, 1, 1, sc),
            "b21c": _conv_lhsT(g("bt21c"), 64, 1, 1, sc),
        }
        self.conv_order = list(self.wconv.keys())
        cmats = []
        for k in self.conv_order:
            c = self.wconv[k]
            m = np.zeros((128, c.shape[1]), np.float32)
            m[:64] = c
            m[64:] = c
            cmats.append(m)
        self.wconv_mat = np.concatenate(cmats, axis=1)

        self.wones = {w: _ones_block(w) for w in (8, 16, 32, 64)}
        self.ones_order = [8, 16, 32, 64]
        self.wones_mat = np.concatenate(
            [self.wones[w] for w in self.ones_order], axis=1)

        self.bias = {}
        border = []
        for d in ("f", "b"):
            for lvl in (1, 2, 3, 4):
                b = g(f"{d}b{lvl}")
                first = (d == "f" and lvl == 1) or (d == "b" and lvl == 4)
                last = d == "b" and lvl == 1
                gates = ("i", "g", "o") if first else (("i", "f", "g") if last else GATES)
                for gt in gates:
                    self.bias[f"{d}{lvl}_{gt}"] = _bias_pair(b, lvl, gt)
                    border.append(f"{d}{lvl}_{gt}")
        self.bias_order = border
        self.bias_mat = np.stack([self.bias[k] for k in border], axis=1)

        self.wih_off = {k: 64 * i for i, k in enumerate(order)}
        self.whh_off = {k: 64 * i for i, k in enumerate(horder)}
        off = {}
        c = 0
        for k in self.conv_order:
            off[k] = c
            c += self.wconv[k].shape[1]
        self.conv_off = off
        self.ones_off = {w: 128 * i for i, w in enumerate(self.ones_order)}
        self.bias_off = {k: i for i, k in enumerate(border)}

    def wall(self):
        """Single merged bf16 weight matrix [128, ncols]:
        wih | whh | wconv | wones | bias-as-bf16."""
        bias_bf = self.bias_mat.astype(ml_dtypes.bfloat16).astype(np.float32)
        mats = [self.wih_mat, self.whh_mat, self.wconv_mat, self.wones_mat,
                bias_bf]
        return np.concatenate(mats, axis=1).astype(ml_dtypes.bfloat16)


# column layout of the merged weight tensor (data-independent)
N_WIH = 29 * 64
N_WHH = 23 * 64
CONV_COLS = {"f12h": 32, "f12c": 32, "f23h": 32, "f23c": 32, "f34h": 32,
             "f34c": 32, "b43h": 64, "b43c": 64, "b32h": 64, "b32c": 64,
             "b21h": 64, "b21c": 64}
N_CONV = sum(CONV_COLS.values())
N_ONES = 512
N_BIAS = 29
N_WALL = N_WIH + N_WHH + N_CONV + N_ONES + N_BIAS


# ---------------------------------------------------------------------------
# Numpy mirror of the device program (for validation; PO=0, one tile).

def mirror_forward(pack: Pack, specT, exact=False):
    """specT: [128, n] float32. Returns outT [128, n]."""
    X = specT.astype(np.float32)
    myexp = (lambda y: np.exp(4.0 * y)) if exact else exp4_np
    myrecip = (lambda x: 1.0 / x) if exact else recip_np

    def mm(lhsT, rhs):
        return lhsT.T.astype(np.float32) @ rhs.astype(np.float32)

    def gates_ih(d, lvl, rhs):
        return {gt: mm(pack.wih[f"{d}{lvl}_{gt}"], rhs)
                for gt in GATES
                if f"{d}{lvl}_{gt}" in pack.wih}

    def add_hh(G, d, lvl, hb):
        for gt in list(G):
            G[gt] = G[gt] + mm(pack.whh[f"{d}{lvl}_{gt}"], hb)

    def add_bias(G, d, lvl):
        for gt in list(G):
            G[gt] = G[gt] + pack.bias[f"{d}{lvl}_{gt}"][:64, None]

    def trans(name_h, name_c, h, c, widths):
        eh = myexp(mm(pack.wconv[name_h], h))
        ec = myexp(mm(pack.wconv[name_c], c))
        e = np.concatenate([eh, ec], axis=0)
        ones = pack.wones[widths][: e.shape[0], : e.shape[0]]
        d_bc = mm(ones, e)
        inv = myrecip(d_bc.astype(np.float32))
        nb = e * inv
        half = eh.shape[0]
        return nb[:half], nb[half:]

    G = gates_ih("f", 1, X); add_bias(G, "f", 1)
    sI, sO, tG = _sigmoid(G["i"]), _sigmoid(G["o"]), np.tanh(G["g"])
    c1 = sI * tG
    h1 = sO * np.tanh(c1)
    hb1, cb1 = trans("f12h", "f12c", h1, c1, 32)

    G = gates_ih("f", 2, X); add_hh(G, "f", 2, hb1); add_bias(G, "f", 2)
    cb1d = np.concatenate([cb1, cb1], axis=0)
    t1 = _sigmoid(G["i"]) * np.tanh(G["g"])
    c2 = _sigmoid(G["f"]) * cb1d + t1
    h2 = _sigmoid(G["o"]) * np.tanh(c2)
    hb2, cb2 = trans("f23h", "f23c", h2, c2, 16)

    G = gates_ih("f", 3, X); add_hh(G, "f", 3, hb2); add_bias(G, "f", 3)
    cb2d = np.concatenate([cb2[0:16], cb2[0:16], cb2[16:32], cb2[16:32]], axis=0)
    t1 = _sigmoid(G["i"]) * np.tanh(G["g"])
    c3 = _sigmoid(G["f"]) * cb2d + t1
    h3 = _sigmoid(G["o"]) * np.tanh(c3)
    hb3, cb3 = trans("f34h", "f34c", h3, c3, 8)

    G = gates_ih("f", 4, X); add_hh(G, "f", 4, hb3); add_bias(G, "f", 4)
    cb3d = np.concatenate(
        [cb3[8 * (s // 2):8 * (s // 2) + 8] for s in range(8)], axis=0)
    t1 = _sigmoid(G["i"]) * np.tanh(G["g"])
    c4 = _sigmoid(G["f"]) * cb3d + t1
    h4 = _sigmoid(G["o"]) * np.tanh(c4)

    G = gates_ih("b", 4, X); add_bias(G, "b", 4)
    c4b = _sigmoid(G["i"]) * np.tanh(G["g"])
    h4b = _sigmoid(G["o"]) * np.tanh(c4b)
    hb4, cb4 = trans("b43h", "b43c", h4b, c4b, 16)

    G = gates_ih("b", 3, X); add_hh(G, "b", 3, hb4); add_bias(G, "b", 3)
    t1 = _sigmoid(G["i"]) * np.tanh(G["g"])
    c3b = _sigmoid(G["f"]) * cb4 + t1
    h3b = _sigmoid(G["o"]) * np.tanh(c3b)
    hb3b, cb3b = trans("b32h", "b32c", h3b, c3b, 32)

    G = gates_ih("b", 2, X); add_hh(G, "b", 2, hb3b); add_bias(G, "b", 2)
    t1 = _sigmoid(G["i"]) * np.tanh(G["g"])
    c2b = _sigmoid(G["f"]) * cb3b + t1
    h2b = _sigmoid(G["o"]) * np.tanh(c2b)
    hb2b, cb2b = trans("b21h", "b21c", h2b, c2b, 64)

    G = gates_ih("b", 1, X); add_hh(G, "b", 1, hb2b); add_bias(G, "b", 1)
    c1b = _sigmoid(G["f"]) * cb2b + _sigmoid(G["i"]) * np.tanh(G["g"])

    return np.concatenate([h4, c1b], axis=0)


# ---------------------------------------------------------------------------
# Custom DVE ops.

def _register_op(op):
    import re
    import concourse.dve_ops as dve_ops

    dve_ops.OPS.append(op)
    dve_ops._SUB_OPCODE_FOR_NAME[op.name] = (
        dve_ops._CUSTOM_DVE_ROW_BASE + len(dve_ops.OPS) - 1)
    dve_ops.CUSTOM_DVE_SPECS[op.name] = op.spec
    for ver in ("v3",):
        try:
            op.compile(ver)
        except ValueError as e:
            m = re.search(rf"\({ver}: ([0-9a-f]+)", str(e))
            if not m:
                raise
            op.uops_sha[ver] = m.group(1)
            op.compile(ver)


def _register_exp4():
    import concourse.dve_ops as dve_ops
    from concourse.dve_ops import DveOp
    from concourse.dve_spec import Spec, Src0, C0, C1, C2, One

    for op in dve_ops.OPS:
        if op.name == "EXP4_ANT":
            return op
    u = Src0
    inner = C1 + C2 * u
    inner2 = C0 + u * inner
    p = One + u * inner2
    sq = p * p
    spec = Spec(
        body=sq * sq,
        reference=lambda in0, in1, s0, s1, imm2:
            (1.0 + in0 * (s0 + in0 * (s1 + imm2 * in0))) ** 4,
    )
    op = DveOp("EXP4_ANT", spec, subdim=False, uops_sha={})
    _register_op(op)
    return op


def _register_recip_mul():
    import concourse.dve_ops as dve_ops
    from concourse.dve_ops import DveOp
    from concourse.dve_spec import Spec, Src0, Src1, C0, C1, Bin, AluOp

    for op in dve_ops.OPS:
        if op.name == "RECIP1_MUL_ANT":
            return op

    def _ref(in0, in1, s0, s1, imm2):
        x = np.ascontiguousarray(in0, dtype=np.float32)
        nx = (~x.view(np.int32)).view(np.float32)
        y0 = nx * np.float32(s0)
        y1 = y0 * (np.float32(s1) - x * y0)
        return y1 * in1

    nx = Bin(AluOp.BITWISE_NOT, Src0, Src0)
    y0 = nx * C0
    y1 = y0 * (C1 - Src0 * y0)
    spec = Spec(body=y1 * Src1, reference=_ref)
    op = DveOp("RECIP1_MUL_ANT", spec, subdim=False, uops_sha={})
    _register_op(op)
    return op


def _register_tanh_mul():
    import concourse.dve_ops as dve_ops
    from concourse.dve_ops import DveOp
    from concourse.dve_spec import Spec, Src0, Src1, C0, C1, C2

    for op in dve_ops.OPS:
        if op.name == "TANH_MUL_ANT":
            return op
    u = Src0
    x2 = u * u
    x4 = x2 * x2
    t = u * (C0 + C1 * x2 + C2 * x4)
    spec = Spec(
        body=t * Src1,
        reference=lambda in0, in1, s0, s1, imm2:
            in0 * (s0 + s1 * in0 * in0 + imm2 * in0**4) * in1,
    )
    op = DveOp("TANH_MUL_ANT", spec, subdim=False, uops_sha={})
    _register_op(op)
    return op


# ---------------------------------------------------------------------------
# Device kernel (Bass / Tile).

_BUILD_CACHE = {}

POOLCFG = {"pgf": 3, "pgb": 3, "pc": 1, "pd": 1, "spool": 4, "xpool": 5,
           "xraw": 3, "opool": 2, "group": 4}


def _build(n_samples):
    """Build + compile the Bacc program for one core processing n_samples."""
    key = (n_samples, tuple(sorted(POOLCFG.items())))
    if key in _BUILD_CACHE:
        return _BUILD_CACHE[key]

    import concourse.bass as bass
    import concourse.mybir as mybir
    from concourse import bacc
    from concourse.tile import TileContext
    from concourse.dve_ops import RECIPROCAL_APPROX_FAST, RECIP_APPROX_FAST_CONSTS

    EXP4 = _register_exp4()
    TMUL = _register_tanh_mul()
    RMUL = _register_recip_mul()
    RC = RECIP_APPROX_FAST_CONSTS

    bf16 = mybir.dt.bfloat16
    f32 = mybir.dt.float32
    i8 = mybir.dt.int8
    u8 = mybir.dt.uint8
    AF = mybir.ActivationFunctionType
    Alu = mybir.AluOpType
    Sig, Tanh = AF.Sigmoid, AF.Tanh

    conv_order = list(CONV_COLS.keys())
    conv_off = {}
    c = 0
    for k in conv_order:
        conv_off[k] = c
        c += CONV_COLS[k]

    # merged weight tensor column offsets
    OFF_WIH = 0
    OFF_WHH = N_WIH
    OFF_CONV = N_WIH + N_WHH
    OFF_ONES = OFF_CONV + N_CONV
    OFF_BIAS = OFF_ONES + N_ONES
    ones_off = {8: 0, 16: 128, 32: 256, 64: 384}

    wih_names = []
    for d in ("f", "b"):
        for lvl in (1, 2, 3, 4):
            first = (d == "f" and lvl == 1) or (d == "b" and lvl == 4)
            last = d == "b" and lvl == 1
            gates = ("i", "g", "o") if first else (("i", "f", "g") if last else GATES)
            for gt in gates:
                wih_names.append(f"{d}{lvl}_{gt}")
    wih_off = {k: 64 * i for i, k in enumerate(wih_names)}
    bias_off = {k: i for i, k in enumerate(wih_names)}

    whh_names = [f"f{l}_{g}" for l in (2, 3, 4) for g in GATES]
    whh_names += [f"b{l}_{g}" for l in (3, 2) for g in GATES]
    whh_names += [f"b1_{g}" for g in ("i", "f", "g")]
    whh_off = {k: 64 * i for i, k in enumerate(whh_names)}
    whh_K = {}
    for k in whh_names:
        d, lvl = k[0], int(k[1])
        whh_K[k] = 32 if d == "f" else 64

    npairs = n_samples // (2 * NT)
    nlow = npairs // 2          # pairs with columns in the low-nibble half
    NSPLIT = 8                  # output tensors (parallel d2h streams)
    nq = n_samples // NSPLIT    # columns per outT split tensor

    nc = bacc.Bacc("TRN2", target_bir_lowering=False, debug=False)
    xhi0 = nc.dram_tensor("xhi0", (128, n_samples // 2), i8,
                          kind="ExternalInput")
    xhi1 = nc.dram_tensor("xhi1", (128, n_samples // 2), i8,
                          kind="ExternalInput")
    xnib = nc.dram_tensor("xnib", (128, n_samples // 2), u8,
                          kind="ExternalInput")
    wall_d = nc.dram_tensor("wall", (128, N_WALL), bf16, kind="ExternalInput")
    outq = [nc.dram_tensor(f"outq{i}", (128, nq), i8, kind="ExternalOutput")
            for i in range(NSPLIT)]
    oscale = nc.dram_tensor("oscale", (128, 2 * npairs), f32,
                            kind="ExternalOutput")

    xhi_ap = [xhi0.ap(), xhi1.ap()]
    xnib_ap = xnib.ap()
    outq_ap = [t.ap() for t in outq]
    oscale_ap = oscale.ap()

    with TileContext(nc) as tc:
        with (
            tc.tile_pool(name="wpool", bufs=1) as wpool,
            tc.tile_pool(name="xpool", bufs=POOLCFG["xpool"]) as xpool,
            tc.tile_pool(name="xraw", bufs=POOLCFG["xraw"]) as xraw,
            tc.tile_pool(name="spool", bufs=POOLCFG["spool"]) as spool,
            tc.tile_pool(name="opool", bufs=POOLCFG["opool"]) as opool,
            tc.tile_pool(name="qpool", bufs=4) as qpool,
            tc.tile_pool(name="pgf", bufs=POOLCFG["pgf"], space="PSUM") as pgf,
            tc.tile_pool(name="pgb", bufs=POOLCFG["pgb"], space="PSUM") as pgb,
            tc.tile_pool(name="pc", bufs=POOLCFG["pc"], space="PSUM") as pc,
            tc.tile_pool(name="pd", bufs=POOLCFG["pd"], space="PSUM") as pd,
        ):
            wall_sb = wpool.tile([128, N_WALL], bf16)
            nc.sync.dma_start(out=wall_sb[:], in_=wall_d.ap()[:, :])

            def wih_sl(name):
                o = OFF_WIH + wih_off[name]
                return wall_sb[:, o:o + 64]

            def whh_sl(base, name):
                o = OFF_WHH + whh_off[name]
                K = whh_K[name]
                return wall_sb[base:base + K, o:o + 64]

            def conv_sl(rows, name, width):
                o = OFF_CONV + conv_off[name]
                return wall_sb[rows[0]:rows[1], o:o + width]

            def ones_sl(width):
                o = OFF_ONES + ones_off[width]
                return wall_sb[:, o:o + 128]

            def bias_sl(name):
                o = OFF_BIAS + bias_off[name]
                return wall_sb[:, o:o + 1]

            def exp4(out_ap, in_ap):
                nc.vector._custom_dve(EXP4, out=out_ap, in0=in_ap,
                                      s0=EXP_C1, s1=EXP_C2, imm2=EXP_C3)

            def tanh_mul(c_ap, s_ap, tag, out=None):
                if out is None:
                    out = spool.tile([128, NT], bf16, tag=tag, name=tag)[:]
                nc.vector._custom_dve(TMUL, out=out, in0=c_ap, in1=s_ap,
                                      s0=TANH_A, s1=TANH_B, imm2=TANH_C)
                return out

            def gates_mm(d, lvl, gates, XA, XB, hbA=None, hbB=None):
                """gate -> psum tile [128, NT]; XA/XB are (hi, lo) pairs."""
                XAhi, XAlo = XA
                XBhi, XBlo = XB
                ps = {}
                pool = pgf if d == "f" else pgb
                for gt in gates:
                    name = f"{d}{lvl}_{gt}"
                    p = pool.tile([128, NT], f32, tag="gates", name="gates")
                    w = wih_sl(name)
                    has_hh = hbA is not None and name in whh_off
                    if has_hh:
                        wA = whh_sl(hbA.base_partition(), name)
                        wB = whh_sl(hbB.base_partition(), name)
                    # NB: each half's accumulation group must close before the
                    # other half's opens — PSUM zero-regions are bank-granular.
                    nc.tensor.matmul(p[0:64, :], w, XAhi[:],
                                     start=True, stop=False)
                    nc.tensor.matmul(p[0:64, :], w, XAlo[:],
                                     start=False, stop=not has_hh)
                    if has_hh:
                        nc.tensor.matmul(p[0:64, :], wA, hbA,
                                         start=False, stop=True)
                    nc.tensor.matmul(p[64:128, :], w, XBhi[:],
                                     start=True, stop=False)
                    nc.tensor.matmul(p[64:128, :], w, XBlo[:],
                                     start=False, stop=not has_hh)
                    if has_hh:
                        nc.tensor.matmul(p[64:128, :], wB, hbB,
                                         start=False, stop=True)
                    ps[gt] = p
                return ps

            def act(func, ps_tile, d, lvl, gt):
                name = f"{d}{lvl}_{gt}"
                o = spool.tile([128, NT], bf16, tag=f"a_{gt}")
                nc.scalar.activation(out=o[:], in_=ps_tile[:], func=func,
                                     bias=bias_sl(name), scale=1.0)
                return o

            def tanh_sbuf(t_in, tag):
                o = spool.tile([128, NT], bf16, tag=tag)
                nc.scalar.activation(out=o[:], in_=t_in[:], func=Tanh)
                return o

            def tt(op, a, b, tag=None, out=None):
                if out is None:
                    out = spool.tile([128, NT], bf16, tag=tag, name=tag)[:]
                if op == "mul":
                    nc.vector.tensor_mul(out, a, b)
                else:
                    nc.vector.tensor_add(out, a, b)
                return out

            def quant_store(v, p, which):
                """Quantize tile v [128, NT x2-half layout] to int8 with
                per-row scale; store to outq + oscale column 2p+which."""
                m = qpool.tile([128, 1], f32, tag=f"m{which}")
                nc.vector.tensor_reduce(out=m[:], in_=v, axis=mybir.AxisListType.X,
                                        op=Alu.max, apply_absolute_value=True)
                m2 = qpool.tile([128, 1], f32, tag=f"m2{which}")
                nc.vector.tensor_scalar_max(m2[:], m[:], 1e-8)
                inv = qpool.tile([128, 1], f32, tag=f"inv{which}")
                nc.vector.reciprocal(out=inv[:], in_=m2[:])
                q = qpool.tile([128, NT], i8, tag=f"q{which}")
                nc.vector.tensor_scalar(out=q[:], in0=v, scalar1=inv[:],
                                        scalar2=127.0, op0=Alu.mult,
                                        op1=Alu.mult)
                ti = (2 * p * NT) // nq
                lo = 2 * p * NT - ti * nq
                rows = (0, 64) if which == 0 else (64, 128)
                nc.sync.dma_start(out=outq_ap[ti][rows[0]:rows[1],
                                                  lo:lo + NT],
                                  in_=q[0:64, :])
                nc.sync.dma_start(out=outq_ap[ti][rows[0]:rows[1],
                                                  lo + NT:lo + 2 * NT],
                                  in_=q[64:128, :])
                nc.gpsimd.dma_start(out=oscale_ap[:, 2 * p + which:2 * p + which + 1],
                                    in_=m2[:])

            def trans_fwd(stage, h_pair, c_pair, width):
                e_ps = pc.tile([128, NT], f32, tag="eps")
                nc.tensor.matmul(e_ps[0:32, :], conv_sl((0, 64), stage + "h", 32),
                                 h_pair[0:64, :], start=True, stop=True,
                                 tile_position=(0, 0))
                nc.tensor.matmul(e_ps[32:64, :], conv_sl((0, 64), stage + "c", 32),
                                 c_pair[0:64, :], start=True, stop=True,
                                 tile_position=(0, 32))
                nc.tensor.matmul(e_ps[64:96, :], conv_sl((64, 128), stage + "h", 32),
                                 h_pair[64:128, :], start=True, stop=True,
                                 tile_position=(64, 64))
                nc.tensor.matmul(e_ps[96:128, :], conv_sl((64, 128), stage + "c", 32),
                                 c_pair[64:128, :], start=True, stop=True,
                                 tile_position=(64, 96))
                e = spool.tile([128, NT], bf16, tag="e")
                exp4(e[:], e_ps[:])
                d_ps = pd.tile([128, NT], f32, tag="dps")
                nc.tensor.matmul(d_ps[:], ones_sl(width), e[:],
                                 start=True, stop=True)
                nb = spool.tile([128, NT], bf16, tag="nb_" + stage)
                nc.vector._custom_dve(RMUL, out=nb[:], in0=d_ps[:], in1=e[:],
                                      s0=RC["s0"], s1=RC["s1"])
                return nb

            def trans_bwd(stage, h_pair, c_pair, width):
                nbs = []
                for which in ("A", "B"):
                    e_ps = pc.tile([128, NT], f32, tag="eps")
                    if which == "A":
                        rh, rc = h_pair[0:64, :], c_pair[0:64, :]
                        wrows = (0, 64)
                        c_rows, h_rows = (0, 64), (64, 128)
                    else:
                        rh, rc = h_pair[64:128, :], c_pair[64:128, :]
                        wrows = (64, 128)
                        c_rows, h_rows = (64, 128), (0, 64)
                    nc.tensor.matmul(e_ps[c_rows[0]:c_rows[1], :],
                                     conv_sl(wrows, stage + "c", 64), rc,
                                     start=True, stop=True)
                    nc.tensor.matmul(e_ps[h_rows[0]:h_rows[1], :],
                                     conv_sl(wrows, stage + "h", 64), rh,
                                     start=True, stop=True)
                    e = spool.tile([128, NT], bf16, tag="e")
                    exp4(e[:], e_ps[:])
                    d_ps = pd.tile([128, NT], f32, tag="dps")
                    nc.tensor.matmul(d_ps[:], ones_sl(width), e[:],
                                     start=True, stop=True)
                    nb = spool.tile([128, NT], bf16, tag=f"nb{which}_" + stage)
                    nc.vector._custom_dve(RMUL, out=nb[:], in0=d_ps[:],
                                          in1=e[:], s0=RC["s0"], s1=RC["s1"])
                    nbs.append(nb)
                return nbs

            def fwd_chain(XA, XB, p):
                G = gates_mm("f", 1, ("i", "g", "o"), XA, XB)
                sI = act(Sig, G["i"], "f", 1, "i")
                sO = act(Sig, G["o"], "f", 1, "o")
                tG = act(Tanh, G["g"], "f", 1, "g")
                c1 = tt("mul", sI[:], tG[:], tag="cst")
                h1 = tanh_mul(c1, sO[:], "h")
                yield
                nb12 = trans_fwd("f12", h1, c1, 32)

                cbd = spool.tile([128, NT], bf16, tag="cbd")
                nc.sync.dma_start(out=cbd[0:32, :], in_=nb12[32:64, :])
                nc.gpsimd.dma_start(out=cbd[32:64, :], in_=nb12[32:64, :])
                nc.sync.dma_start(out=cbd[64:96, :], in_=nb12[96:128, :])
                nc.gpsimd.dma_start(out=cbd[96:128, :], in_=nb12[96:128, :])
                yield

                G = gates_mm("f", 2, GATES, XA, XB,
                             nb12[0:32, :], nb12[64:96, :])
                sI = act(Sig, G["i"], "f", 2, "i")
                sF = act(Sig, G["f"], "f", 2, "f")
                sO = act(Sig, G["o"], "f", 2, "o")
                tG = act(Tanh, G["g"], "f", 2, "g")
                t1 = tt("mul", sI[:], tG[:], tag="t1")
                t2 = tt("mul", sF[:], cbd[:], tag="t2")
                c2 = tt("add", t1, t2, tag="cst")
                h2 = tanh_mul(c2, sO[:], "h")
                yield
                nb23 = trans_fwd("f23", h2, c2, 16)

                cbd = spool.tile([128, NT], bf16, tag="cbd")
                for base, src in ((0, 32), (64, 96)):
                    for s in range(4):
                        blk = src + 16 * (s // 2)
                        eng = nc.sync if s % 2 else nc.gpsimd
                        eng.dma_start(
                            out=cbd[base + 16 * s:base + 16 * s + 16, :],
                            in_=nb23[blk:blk + 16, :])
                yield

                G = gates_mm("f", 3, GATES, XA, XB,
                             nb23[0:32, :], nb23[64:96, :])
                sI = act(Sig, G["i"], "f", 3, "i")
                sF = act(Sig, G["f"], "f", 3, "f")
                sO = act(Sig, G["o"], "f", 3, "o")
                tG = act(Tanh, G["g"], "f", 3, "g")
                t1 = tt("mul", sI[:], tG[:], tag="t1")
                t2 = tt("mul", sF[:], cbd[:], tag="t2")
                c3 = tt("add", t1, t2, tag="cst")
                h3 = tanh_mul(c3, sO[:], "h")
                yield
                nb34 = trans_fwd("f34", h3, c3, 8)

                cbd = spool.tile([128, NT], bf16, tag="cbd")
                for base, src in ((0, 32), (64, 96)):
                    for s in range(8):
                        blk = src + 8 * (s // 2)
                        eng = nc.sync if s % 2 else nc.gpsimd
                        eng.dma_start(
                            out=cbd[base + 8 * s:base + 8 * s + 8, :],
                            in_=nb34[blk:blk + 8, :])
                yield

                G = gates_mm("f", 4, GATES, XA, XB,
                             nb34[0:32, :], nb34[64:96, :])
                sI = act(Sig, G["i"], "f", 4, "i")
                sF = act(Sig, G["f"], "f", 4, "f")
                sO = act(Sig, G["o"], "f", 4, "o")
                tG = act(Tanh, G["g"], "f", 4, "g")
                t1 = tt("mul", sI[:], tG[:], tag="t1")
                t2 = tt("mul", sF[:], cbd[:], tag="t2")
                c4 = tt("add", t1, t2, tag="cst")
                tC = tanh_sbuf(c4, "tC")
                outh = opool.tile([128, NT], bf16, tag="outh")
                tt("mul", sO[:], tC[:], out=outh[:])
                quant_store(outh[:], p, 0)

            def bwd_chain(XA, XB, p):
                G = gates_mm("b", 4, ("i", "g", "o"), XA, XB)
                sI = act(Sig, G["i"], "b", 4, "i")
                sO = act(Sig, G["o"], "b", 4, "o")
                tG = act(Tanh, G["g"], "b", 4, "g")
                c4b = tt("mul", sI[:], tG[:], tag="cstb")
                h4b = tanh_mul(c4b, sO[:], "hb")
                yield
                nbA, nbB = trans_bwd("b43", h4b, c4b, 16)
                yield

                G = gates_mm("b", 3, GATES, XA, XB,
                             nbA[64:128, :], nbB[0:64, :])
                sI = act(Sig, G["i"], "b", 3, "i")
                sF = act(Sig, G["f"], "b", 3, "f")
                sO = act(Sig, G["o"], "b", 3, "o")
                tG = act(Tanh, G["g"], "b", 3, "g")
                t1 = tt("mul", sI[:], tG[:], tag="t1b")
                t2b = spool.tile([128, NT], bf16, tag="t2b", name="t2b")
                nc.vector.tensor_mul(t2b[0:64, :], sF[0:64, :],
                                     nbA[0:64, :])
                nc.vector.tensor_mul(t2b[64:128, :], sF[64:128, :],
                                     nbB[64:128, :])
                c3b = tt("add", t1, t2b[:], tag="cstb")
                h3b = tanh_mul(c3b, sO[:], "hb")
                yield
                nbA, nbB = trans_bwd("b32", h3b, c3b, 32)
                yield

                G = gates_mm("b", 2, GATES, XA, XB,
                             nbA[64:128, :], nbB[0:64, :])
                sI = act(Sig, G["i"], "b", 2, "i")
                sF = act(Sig, G["f"], "b", 2, "f")
                sO = act(Sig, G["o"], "b", 2, "o")
                tG = act(Tanh, G["g"], "b", 2, "g")
                t1 = tt("mul", sI[:], tG[:], tag="t1b")
                t2b = spool.tile([128, NT], bf16, tag="t2b", name="t2b")
                nc.vector.tensor_mul(t2b[0:64, :], sF[0:64, :],
                                     nbA[0:64, :])
                nc.vector.tensor_mul(t2b[64:128, :], sF[64:128, :],
                                     nbB[64:128, :])
                c2b = tt("add", t1, t2b[:], tag="cstb")
                h2b = tanh_mul(c2b, sO[:], "hb")
                yield
                nbA, nbB = trans_bwd("b21", h2b, c2b, 64)
                yield

                G = gates_mm("b", 1, ("i", "f", "g"), XA, XB,
                             nbA[64:128, :], nbB[0:64, :])
                sI = act(Sig, G["i"], "b", 1, "i")
                sF = act(Sig, G["f"], "b", 1, "f")
                tG = act(Tanh, G["g"], "b", 1, "g")
                t1 = tt("mul", sI[:], tG[:], tag="t1b")
                t2b = spool.tile([128, NT], bf16, tag="t2b", name="t2b")
                nc.vector.tensor_mul(t2b[0:64, :], sF[0:64, :],
                                     nbA[0:64, :])
                nc.vector.tensor_mul(t2b[64:128, :], sF[64:128, :],
                                     nbB[64:128, :])
                outc = opool.tile([128, NT], bf16, tag="outc")
                tt("add", t1, t2b[:], out=outc[:])
                quant_store(outc[:], p, 1)

            import itertools

            def decode_half(tag, hi_cols, nib_cols, high_nibble):
                """Load + decode one NT-column half: returns (Xhi, Xlo) bf16."""
                hi8 = xraw.tile([128, NT], i8, tag=f"hi8{tag}")
                nc.sync.dma_start(out=hi8[:],
                                  in_=xhi_ap[0 if high_nibble == 0 else 1][:, nib_cols])
                nib = xraw.tile([128, NT], u8, tag=f"nib{tag}")
                nc.sync.dma_start(out=nib[:], in_=xnib_ap[:, nib_cols])
                Xhi = xpool.tile([128, NT], bf16, tag=f"Xhi{tag}")
                nc.scalar.activation(out=Xhi[:], in_=hi8[:], func=AF.Copy)
                lo8 = xraw.tile([128, NT], u8, tag=f"lo8{tag}")
                if high_nibble:
                    nc.vector.tensor_scalar(out=lo8[:], in0=nib[:], scalar1=4,
                                            op0=Alu.logical_shift_right,
                                            scalar2=None)
                else:
                    nc.vector.tensor_scalar(out=lo8[:], in0=nib[:], scalar1=15,
                                            op0=Alu.bitwise_and, scalar2=None)
                Xlo = xpool.tile([128, NT], bf16, tag=f"Xlo{tag}")
                nc.scalar.activation(out=Xlo[:], in_=lo8[:], func=AF.Copy,
                                     scale=0.0625)
                return Xhi, Xlo

            def pair_chains(p):
                high = 1 if p >= nlow else 0
                pn = p - nlow if high else p
                nA = slice(2 * pn * NT, 2 * pn * NT + NT)
                nB = slice(2 * pn * NT + NT, 2 * pn * NT + 2 * NT)
                XA = decode_half("A", None, nA, high)
                XB = decode_half("B", None, nB, high)
                return (fwd_chain(XA, XB, p), bwd_chain(XA, XB, p))

            GROUP = POOLCFG.get("group", 1)
            for p0 in range(0, npairs, GROUP):
                chains = []
                for p in range(p0, min(p0 + GROUP, npairs)):
                    chains.extend(pair_chains(p))
                for _ in itertools.zip_longest(*chains):
                    pass

    nc.compile()
    _BUILD_CACHE[key] = nc
    return nc


# ---------------------------------------------------------------------------
# Fast single-core runner: cached jit around the bass_exec primitive.

_RUNNER_CACHE = {}


def _make_runner(nc):
    if id(nc) in _RUNNER_CACHE:
        return _RUNNER_CACHE[id(nc)]

    import jax
    import jax.numpy as jnp
    import concourse.mybir as mybir
    from concourse import bass2jax

    bass2jax.install_neuronx_cc_hook()

    in_names, out_names, out_avals = [], [], []
    pname = nc.partition_id_tensor.name if nc.partition_id_tensor else None
    for alloc in nc.m.functions[0].allocations:
        if not isinstance(alloc, mybir.MemoryLocationSet):
            continue
        name = alloc.memorylocations[0].name
        if alloc.kind == "ExternalInput":
            if name != pname:
                in_names.append(name)
        elif alloc.kind == "ExternalOutput":
            out_names.append(name)
            out_avals.append(jax.core.ShapedArray(
                tuple(alloc.tensor_shape), mybir.dt.np(alloc.dtype)))

    all_names = list(in_names) + list(out_names)
    if pname is not None:
        all_names.append(pname)

    def _body(*args):
        operands = list(args)
        if pname is not None:
            operands.append(bass2jax.partition_id_tensor())
        outs = bass2jax._bass_exec_p.bind(
            *operands,
            out_avals=tuple(out_avals),
            in_names=tuple(all_names),
            out_names=tuple(out_names),
            lowering_input_output_aliases=(),
            sim_require_finite=True,
            sim_require_nnan=True,
            nc=nc,
        )
        return tuple(outs)

    jitted = jax.jit(_body, keep_unused=True)
    # device-resident zero buffers for the output operands: created once on
    # device (never shipped over the tunnel), reused read-only every call.
    dev = jax.devices()[0]
    zeros = jax.jit(
        lambda: tuple(jnp.zeros(a.shape, a.dtype) for a in out_avals),
        device=dev)()
    runner = (jitted, in_names, out_names, list(zeros))
    _RUNNER_CACHE[id(nc)] = runner
    return runner


def _run_device(nc, in_map):
    """Threaded upload -> single jit call -> threaded fetch.

    Returns dict name -> np.ndarray. This is the full device round trip
    (host arrays in, host arrays out) for one run.
    """
    import jax

    jitted, in_names, out_names, zeros = _make_runner(nc)
    dev = jax.devices()[0]

    # threaded upload of all inputs
    staged = {}
    errs = []

    def run_safe(f, *a):
        try:
            f(*a)
        except Exception as e:          # propagate from worker threads
            errs.append(e)

    def put(name):
        x = jax.device_put(in_map[name], dev)
        x.block_until_ready()
        staged[name] = x

    threads = [threading.Thread(target=run_safe, args=(put, n))
               for n in in_names]
    for t in threads:
        t.start()
    for t in threads:
        t.join()
    if errs:
        raise errs[0]

    outs = jitted(*[staged[n] for n in in_names], *zeros)
    res = {}

    def get(i, name):
        res[name] = np.asarray(outs[i])

    threads = [threading.Thread(target=run_safe, args=(get, i, n))
               for i, n in enumerate(out_names)]
    for t in threads:
        t.start()
    for t in threads:
        t.join()
    if errs:
        raise errs[0]
    return res


# ---------------------------------------------------------------------------
# Host quantize / dequantize.

def quantize_input(spec):
    """spec [B, D] f32 -> (xhi0, xhi1 int8 [128, B//2], xnib u8 [128, B//2],
    amax)."""
    spec = np.asarray(spec, np.float32)
    amax = float(np.abs(spec).max())
    n = spec.shape[0]
    xT = spec.T                                  # [128, B] view
    q = np.rint(xT * (QBITS_MAX / amax)).astype(np.int32)
    np.clip(q, -QBITS_MAX, QBITS_MAX, out=q)
    hi = (q >> 4).astype(np.int8)
    lo = (q & 15).astype(np.uint8)
    n2 = n // 2
    xnib = (lo[:, :n2] | (lo[:, n2:] << 4)).astype(np.uint8)
    return (np.ascontiguousarray(hi[:, :n2]), np.ascontiguousarray(hi[:, n2:]),
            np.ascontiguousarray(xnib), amax)


def dequantize_output(res, n):
    """res: dict with outq0..7 int8 [128, n//8] + oscale f32 [128, 2*npairs].
    Returns [n, 128] float32."""
    npairs = n // (2 * NT)
    q = np.concatenate([np.asarray(res[f"outq{i}"]) for i in range(8)], axis=1)
    sc = np.asarray(res["oscale"], np.float32)      # [128, 2*npairs]
    mh = sc[:, 0::2]                                # [128, npairs]
    mc = sc[:, 1::2]
    # element [r, pair p, half h, col] has scale:
    #   r <  64: mh[64*h + r, p]        (outh rows)
    #   r >= 64: mc[64*h + (r-64), p]   (outc rows)
    S = np.empty((128, npairs, 2), np.float32)
    for h in (0, 1):
        S[:64, :, h] = mh[64 * h:64 * h + 64, :]
        S[64:, :, h] = mc[64 * h:64 * h + 64, :]
    out = q.astype(np.float32).reshape(128, npairs, 2, NT)
    out *= (S / 127.0)[:, :, :, None]
    return np.ascontiguousarray(out.reshape(128, n).T)


def _bf16(a):
    return np.ascontiguousarray(a).astype(ml_dtypes.bfloat16)


def prepare_inputs(inputs):
    """Host prep: pack weights + quantize spec. Returns (nc, in_map, n)."""
    spec = np.asarray(inputs["spec"], np.float32)
    n = spec.shape[0]
    xhi0, xhi1, xnib, amax = quantize_input(spec)
    # x_hat = (amax/127) * (hi + lo/16); fold amax/127 into wih
    pack = Pack(inputs, xscale=amax / 127.0)
    nc = _build(n)
    in_map = {"xhi0": xhi0, "xhi1": xhi1, "xnib": xnib,
              "wall": np.ascontiguousarray(pack.wall())}
    return nc, in_map, n


def kernel(**inputs):
    nc, in_map, n = prepare_inputs(inputs)
    res = _run_device(nc, in_map)
    return dequantize_output(res, n)


if __name__ == "__main__":
    pass
